# revision 17
# baseline (speedup 1.0000x reference)
"""Block-global self-attention Trainium2 kernel (SPMD over 8 NeuronCores).

Sharding: core c -> batch n = c//4, heads h0 = (c%4)*4 .. h0+3.
Each core receives xt = hidden[n].T (bf16, chunk-major) and wq/wk/wv =
W[:, cols] [2048,512] bf16, returns out [4096,512] (its head-column
stripe of batch n).

Per-core pipeline:
  P: bf16 projections (direct DMA of host-side bf16 xt/weights) -> qT/kT
     [d,t] + V2 (t-major, 64-row-shifted so local windows are two aligned
     full-K tiles). Approx q-norms land in a [128,NH,32] grid per chunk
     (DRAM bounce per chunk); the packed value (quantized norm + token id
     in low mantissa) transform + a DVE 32x32 stream-transpose into pkT
     [(h,j), pair, p] run incrementally under the chunk loop. Local
     blocks interleave with a 1-chunk lag; the last DEFER blocks are
     deferred to cover phase-B latency.
  A: local block attention; softmax without max-subtraction (|score|<8);
     probs kept unnormalized bf16, 1/denom fused into the final ACT copy.
  B: top-16 per pkT row (128-token classes) -> PE transpose + one DRAM
     bounce -> per-head pools [4,512] -> 9 max8/match_replace rounds ->
     top-72 candidates + bos/eos. Four indirect row gathers (hi|lo bf16
     pairs from host-packed xhl) issue back-to-back and overlap the
     deferred local blocks. Exact fp32 candidate q (hi/lo trick, wlo
     host-precomputed) via DMA-transposed slabs (no PE transposes);
     global attention transposed over all 74 slots; exact top-62
     threshold picks the final set; per-head scatters roll out as soon
     as selection + that head's ctx are ready.
"""
import os
import numpy as np

import concourse.bass as bass
import concourse.bacc as bacc
import concourse.mybir as mybir
from concourse.tile import TileContext, add_dep_helper
from concourse.bass_utils import run_bass_kernel_spmd

F32 = mybir.dt.float32
BF16 = mybir.dt.bfloat16
I32 = mybir.dt.int32

T = 4096
H = 2048
D = 128
NH = 4
KO = H // 128
NB = T // 128
CW = 512
NCHUNK = T // CW
NEG = -30.0
SCALE = float(1.0 / np.sqrt(128.0))
NCAND = 72
NSLOT = NCAND + 2
NPAD = 80          # gather/transpose row padding (xbar needs %16)
GEXP = 512 // NSLOT  # global score blocks per psum bank / exp call
DEBUG = bool(int(os.environ.get("KERNEL_DEBUG", "0")))
# PE rest: chained delay-DMAs per chunk boundary; breaks the sustained
# PE-activity streak that trips the P0 power-state downclock
REST = int(os.environ.get("KERNEL_REST", "1"))
DEFER = int(os.environ.get("KERNEL_DEFER", "16"))
REST_B = int(os.environ.get("KERNEL_REST_B", "1"))


def ts(i, sz):
    return slice(i * sz, (i + 1) * sz)


def _raw(inst):
    return inst.ins if hasattr(inst, "ins") else inst


def build_program():
    nc = bacc.Bacc("TRN2", target_bir_lowering=False, debug=False,
                   enable_asserts=True)
    # chunk-major xt layout: [c, ko, p, t] so each chunk DMA reads a
    # contiguous 512KB slab (sequential DRAM >> strided)
    xt_d = nc.dram_tensor("xt", (NCHUNK, KO, 128, CW), BF16,
                          kind="ExternalInput").ap()
    xhl_d = nc.dram_tensor("xhl", (T, 2 * H), BF16, kind="ExternalInput").ap()
    wq_d = nc.dram_tensor("wq", (H, NH * D), BF16, kind="ExternalInput").ap()
    wk_d = nc.dram_tensor("wk", (H, NH * D), BF16, kind="ExternalInput").ap()
    wv_d = nc.dram_tensor("wv", (H, NH * D), BF16, kind="ExternalInput").ap()
    wlo_d = nc.dram_tensor("wlo", (H, NH * D), BF16, kind="ExternalInput").ap()
    id_d = nc.dram_tensor("ident", (128, 128), F32, kind="ExternalInput").ap()
    idb_d = nc.dram_tensor("identb", (128, 128), BF16, kind="ExternalInput").ap()
    out_d = nc.dram_tensor("out", (T, NH * D), F32, kind="ExternalOutput").ap()
    dbg = {}
    if DEBUG:
        dbg["na"] = nc.dram_tensor("dbg_na", (128, NH, 32), F32, kind="ExternalOutput").ap()
        dbg["cand"] = nc.dram_tensor("dbg_cand", (NH, NSLOT), F32, kind="ExternalOutput").ap()
        dbg["ne"] = nc.dram_tensor("dbg_ne", (NH, NSLOT), F32, kind="ExternalOutput").ap()
        dbg["sidx"] = nc.dram_tensor("dbg_sidx", (NSLOT, NH), I32, kind="ExternalOutput").ap()

    with TileContext(nc) as tc:
        const = tc.alloc_tile_pool(name="const", bufs=1)
        res = tc.alloc_tile_pool(name="res", bufs=1)
        dram = tc.alloc_tile_pool(name="dram", bufs=1, space="DRAM")

        ident = const.tile([128, 128], F32)
        nc.sync.dma_start(ident[:], id_d)
        identb = const.tile([128, 128], BF16)
        nc.sync.dma_start(identb[:], idb_d)
        ones_b = const.tile([128, 1], BF16)
        nc.vector.memset(ones_b[:], 1.0)
        ones = const.tile([128, 1], F32)
        nc.vector.memset(ones[:], 1.0)
        iota_g = const.tile([128, NH, 32], F32)
        nc.gpsimd.iota(iota_g[:], pattern=[[0, NH], [1, 32]], base=0,
                       channel_multiplier=32, allow_small_or_imprecise_dtypes=True)
        kT = [res.tile([128, 64 + T + 64], BF16, tag=f"kT{h}", name=f"kT{h}") for h in range(NH)]
        V2 = res.tile([128, NB + 1, NH, D + 1], BF16, tag="V2")
        pkT = res.tile([128, NCHUNK // 2, 32], F32, tag="pkT")
        wqb = res.tile([128, KO, NH * D], BF16, tag="wqb")
        na_dram = dram.tile([NH, T], F32)

        # ---------------- pools ----------------
        psum = tc.alloc_tile_pool(name="psum", bufs=1, space="PSUM")
        ab = tc.alloc_tile_pool(name="ab", bufs=4)

        def psA2k(nm):   # 2KB f32 one-shot psums
            t = psum.tile([128, 512], F32, tag="A2k", bufs=2, name=nm)
            return t
        def psBLK(nm):   # per-block S + ctx combined
            t = psum.tile([128, 512], F32, tag="blk", bufs=2, name=nm)
            return t
        def psSG(nm):    # global score groups
            t = psum.tile([128, 512], F32, tag="psg", bufs=2, name=nm)
            return t
        def psACC(nm):   # held accumulators
            t = psum.tile([128, 512], F32, tag="ACC", bufs=2, name=nm)
            return t

        # ---------------- interleaved: local attention + global per head ----------------
        out_write_insts = []
        cur_co = [None]
        rest_gate = [None]

        def local_block(h, b):
            blk = psBLK("blk")
            # S^T halves: [tk(128), tq(128)]; half g covers window pos g*128..,
            # i.e. k tokens [b*128 - 64 + g*128, ...). kT is 64-padded.
            for g in range(2):
                seg = b + g
                mi = nc.tensor.matmul(blk[:, g * 128:(g + 1) * 128],
                                 kT[h][:, seg * 128:seg * 128 + 128],
                                 qT[h][:, ts(b, 128)], start=True, stop=True)
                if rest_gate[0] is not None:
                    add_dep_helper(_raw(mi), rest_gate[0], reason="rest gate")
                    rest_gate[0] = None
            PT = ab.tile([128, 256], BF16, tag="PT", name="PT", bufs=2)
            nc.scalar.activation(PT[:], blk[:, 0:256], mybir.ActivationFunctionType.Exp,
                                 scale=SCALE)
            pC = blk[:, 256:385]
            nc.tensor.matmul(pC, PT[:, 0:128], V2[:, b, h, :],
                             start=True, stop=False)
            nc.tensor.matmul(pC, PT[:, 128:256], V2[:, b + 1, h, :],
                             start=False, stop=True)
            rc = ab.tile([128, 1], F32, tag="rc", name="rc", bufs=8)
            nc.vector.reciprocal(rc[:], pC[:, 128:129])
            # all 4 heads of a block share one staging tile -> one 256KB
            # out write with 2KB rows (descriptor-rate-bound: 4x fewer DMAs)
            if h == 0:
                cur_co[0] = ab.tile([128, NH, D], F32, tag="co4", name="co4",
                                    bufs=2)
            nc.vector.tensor_scalar_mul(cur_co[0][:, h, :], pC[:, 0:D], rc[:])
            if h == NH - 1:
                w = nc.sync.dma_start(
                    out_d[ts(b, 128), :],
                    cur_co[0][:].rearrange("p h d -> p (h d)"))
                out_write_insts.append(_raw(w))

        def global_scores(h):
            # SgT blocks: psum [t(128), slot]; block jj covers tokens
            # jj*128-64 .. jj*128+63 (kT cols jj*128..+128, V2 block jj).
            # Pad tokens give exp(0)=1 but V2 values AND ones-col are 0
            # there, so they contribute nothing.
            PgT = gbig.tile([128, NB + 1, NSLOT], BF16, tag="PgT",
                            name=f"PgT{h}", bufs=2)
            jj = 0
            while jj <= NB:
                nb = min(GEXP, NB + 1 - jj)
                psg = psSG("psg")
                for gi in range(nb):
                    nc.tensor.matmul(psg[:, gi * NSLOT:(gi + 1) * NSLOT],
                                     kT[h][:, (jj + gi) * 128:(jj + gi + 1) * 128],
                                     qgTh[h][:], start=True, stop=True)
                nc.scalar.activation(
                    PgT[:, jj:jj + nb, :],
                    psg[:, 0:nb * NSLOT].rearrange("p (b s) -> p b s", b=nb),
                    mybir.ActivationFunctionType.Exp, scale=SCALE)
                jj += nb
            return PgT

        def global_ctx(h, PgT):
            pgc = psACC("pgc")[:NSLOT, :D + 1]
            for jj in range(NB + 1):
                nc.tensor.matmul(pgc, PgT[:, jj, :], V2[:, jj, h, :],
                                 start=(jj == 0), stop=(jj == NB),
                                 skip_group_check=True)
            rcg = gw.tile([NSLOT, 1], F32, tag="rcg", bufs=4)
            nc.vector.reciprocal(rcg[:], pgc[:, D:D + 1])
            gco = gw.tile([NSLOT, 128], F32, tag="gco", bufs=4)
            nc.vector.tensor_scalar_mul(gco[:], pgc[:, 0:D], rcg[:])
            return gco

        def scatter_head(h, gco):
            # out viewed as [T*NH, D] rows; sidx encodes token*NH + h so the
            # out AP keeps offset 0 (DynamicAP requirement)
            scat = nc.gpsimd.indirect_dma_start(
                out=out_d.rearrange("t (h d) -> (t h) d", h=NH),
                out_offset=bass.IndirectOffsetOnAxis(ap=sidx_i[:, h:h + 1], axis=0),
                in_=gco[:], in_offset=None,
                bounds_check=T * NH - 1, oob_is_err=False)
            for w in out_write_insts:
                add_dep_helper(_raw(scat), w, reason="scatter after local writes")


        A_DONE = [0]
        # ---------------- phase P ----------------
        wkv2 = tc.alloc_tile_pool(name="wkv2", bufs=1)
        wkv = tc.alloc_tile_pool(name="wkv", bufs=1)
        qT = [wkv2.tile([128, T], BF16, tag=f"qT{h}", name=f"qT{h}") for h in range(NH)]
        wkb = wkv.tile([128, KO, NH * D], BF16, tag="wkb")
        wvb = wkv.tile([128, KO, NH * D], BF16, tag="wvb")
        wb = {"q": wqb, "k": wkb, "v": wvb}

        with tc.tile_pool(name="pp", bufs=2) as pp, \
             tc.tile_pool(name="pp1", bufs=1) as pp1:

            xtb_tiles = {}

            def load_xtb(c):
                t = pp1.tile([128, KO, CW], BF16, tag="xtb", bufs=2)
                for kg in range(4):
                    nc.gpsimd.dma_start(
                        t[:, kg * 4:(kg + 1) * 4, :],
                        xt_d[c, kg * 4:(kg + 1) * 4, :, :].rearrange("ko p t -> p ko t"))
                return t

            # ramp order on the Pool queue: wq -> x chunk 0 -> wk -> wv
            # (queues serialize at the DMA arbiter, so issue in need-order)
            wrs = {nm: wd.rearrange("(ko p) m -> p ko m", p=128)
                   for nm, wd in (("q", wq_d), ("k", wk_d), ("v", wv_d))}
            wlor = wlo_d.rearrange("(ko p) m -> p ko m", p=128)
            nc.gpsimd.dma_start(wb["q"][:], wrs["q"][:])
            xtb_tiles[0] = load_xtb(0)
            for nm in ("k", "v"):
                nc.gpsimd.dma_start(wb[nm][:], wrs[nm][:])

            for h in range(NH):
                nc.vector.memset(kT[h][:, 0:64], 0.0)
                nc.vector.memset(kT[h][:, 64 + T:], 0.0)
            nc.vector.memset(V2[0:64, 0, :, :], 0.0)
            nc.vector.memset(V2[64:128, NB, :, :], 0.0)
            nc.vector.memset(V2[:, :, :, D:D + 1], 1.0)
            # pad rows contribute neither value nor denominator mass
            nc.vector.memset(V2[0:64, 0, :, D:D + 1], 0.0)
            nc.vector.memset(V2[64:128, NB, :, D:D + 1], 0.0)

            # packed-value transform scratch (slab-sliced per chunk)
            m0g = pp.tile([128, NH, 32], F32, tag="m0g", bufs=1)
            m1g = pp.tile([128, NH, 32], F32, tag="m1g", bufs=1)
            nagpg = pp.tile([128, NH, 32], F32, tag="nagpg", bufs=1)
            pkg = pp.tile([128, NH, 32], F32, tag="pkg", bufs=1)
            pkig = pp.tile([128, NH, 32], I32, tag="pkig", bufs=1)
            pkg2 = pkg[:].rearrange("p h j -> p (h j)")

            for c in range(NCHUNK):
                xtb = xtb_tiles.pop(c) if c in xtb_tiles else load_xtb(c)
                na_chunk_writes = []
                for h in range(NH):
                    for nm, dstT in (("q", qT[h]), ("k", kT[h])):
                        ps = psA2k("psqk")
                        for kb in range(KO):
                            mi = nc.tensor.matmul(ps[:], wb[nm][:, kb, ts(h, D)],
                                                  xtb[:, kb, :], start=(kb == 0),
                                                  stop=(kb == KO - 1))
                            if rest_gate[0] is not None:
                                add_dep_helper(_raw(mi), rest_gate[0],
                                               reason="PE rest gate")
                                rest_gate[0] = None
                        off = 64 if nm == "k" else 0
                        nc.vector.tensor_copy(dstT[:, off + c * CW:off + (c + 1) * CW], ps[:])
                        if nm == "q":
                            sq = pp.tile([128, CW], BF16, tag="sq", bufs=1)
                            nc.vector.tensor_tensor(sq[:], dstT[:, ts(c, CW)],
                                                    dstT[:, ts(c, CW)],
                                                    op=mybir.AluOpType.mult)
                            pn = psA2k("pn")[:1, :]
                            nc.tensor.matmul(pn, ones_b[:], sq[:],
                                             start=True, stop=True)
                            narow = pp.tile([1, CW], F32, tag="narow", bufs=1)
                            nc.vector.tensor_copy(narow[:], pn)
                            w = nc.sync.dma_start(na_dram[h:h + 1, ts(c, CW)], narow[:])
                            na_chunk_writes.append(_raw(w))
                for s in range(CW // 128):
                    sg = c * (CW // 128) + s
                    pv = psA2k("psv")
                    for kb in range(KO):
                        nc.tensor.matmul(pv[:], xtb[:, kb, ts(s, 128)],
                                         wb["v"][:, kb, :], start=(kb == 0),
                                         stop=(kb == KO - 1))
                    vt = pp.tile([128, NH * D], BF16, tag="vtmp", bufs=1)
                    nc.vector.tensor_copy(vt[:], pv[:])
                    nc.sync.dma_start(V2[64:128, sg, :, 0:D],
                                      vt[0:64, :].rearrange("p (h d) -> p h d", h=NH))
                    nc.sync.dma_start(V2[0:64, sg + 1, :, 0:D],
                                      vt[64:128, :].rearrange("p (h d) -> p h d", h=NH))
                # incremental norm grid + packed transform for this chunk
                # (tokens c*512.. live on grid partitions c*16..c*16+16)
                r = nc.sync.dma_start(
                    nagpg[ts(c, 16), :, :],
                    na_dram[:, ts(c, CW)].rearrange("h (p j) -> p h j", p=16))
                for w in na_chunk_writes:
                    add_dep_helper(_raw(r), w, reason="na slab read after writes")
                if c % 2 == 1:
                    # DVE partition offsets are quadrant-granular: transform
                    # the finished 32-partition chunk pair, then
                    # stream-transpose it into pkT rows (h*32+j)
                    t2 = c // 2
                    S = slice(t2 * 32, (t2 + 1) * 32)
                    nc.vector.tensor_scalar(m0g[S], iota_g[S], 0.0, scalar2=None,
                                            op0=mybir.AluOpType.is_equal)
                    nc.vector.tensor_scalar(m1g[S], iota_g[S], 4095.0, scalar2=None,
                                            op0=mybir.AluOpType.is_equal)
                    nc.vector.tensor_tensor(m0g[S], m0g[S], m1g[S], op=mybir.AluOpType.add)
                    nc.vector.tensor_tensor(m1g[S], nagpg[S], m0g[S], op=mybir.AluOpType.mult)
                    nc.vector.tensor_tensor(nagpg[S], nagpg[S], m1g[S], op=mybir.AluOpType.subtract)
                    nc.vector.tensor_scalar_mul(m0g[S], m0g[S], 1.0e6)
                    nc.vector.tensor_tensor(nagpg[S], nagpg[S], m0g[S], op=mybir.AluOpType.subtract)
                    nc.vector.tensor_scalar_mul(pkg[S], nagpg[S], 4.0)
                    nc.vector.tensor_copy(pkig[S], pkg[S])
                    nc.vector.tensor_copy(pkg[S], pkig[S])
                    nc.vector.tensor_scalar_mul(pkg[S], pkg[S], 0.125)
                    nc.vector.tensor_scalar_mul(m1g[S], iota_g[S], 2.0 ** -16)
                    nc.vector.tensor_tensor(pkg[S], pkg[S], m1g[S], op=mybir.AluOpType.add)
                    for jb in range(4):
                        nc.vector.transpose(pkT[ts(jb, 32), t2, :],
                                            pkg2[S, ts(jb, 32)])
                # interleave ready local-attention blocks (1-chunk lag);
                # hold back the last blocks to cover phase-B latency
                hi = min(4 * c - 2 + 1, NB - DEFER)
                for b in range(A_DONE[0], hi):
                    for h in range(NH):
                        local_block(h, b)
                A_DONE[0] = max(A_DONE[0], hi)
                if REST and c < NCHUNK - 1:
                    last = None
                    for rr in range(REST):
                        rd = dram.tile([128, 4, CW], BF16, tag="restd")
                        w = nc.gpsimd.dma_start(
                            rd[:], xt_d[c, 0:4, :, :].rearrange("ko p t -> p ko t"))
                        if last is not None:
                            add_dep_helper(_raw(w), last, reason="rest chain")
                        last = _raw(w)
                    rest_gate[0] = last

        wkv.release()

        # ---------------- phase B part 1: candidate top-72 funnel ----------------
        gp = tc.alloc_tile_pool(name="gp", bufs=1)
        gbig = tc.alloc_tile_pool(name="gbig", bufs=2)
        gw = tc.alloc_tile_pool(name="gw", bufs=2)
        # wq residual for the exact re-projection; only used in phase B so
        # loaded here (after wkv released its SBUF), overlapping the funnel
        wlo = gbig.tile([128, KO, NH * D], BF16, tag="wlo", bufs=1)
        nc.scalar.dma_start(wlo[:], wlor[:])

        # top-16 per pkT row (row = (h,j): 128 tokens {p*32+j}); top-72 of a
        # head has <=16 tokens in any such class w.h.p.
        pkT2 = pkT[:].rearrange("p t j -> p (t j)")
        m16 = gp.tile([128, 16], F32)
        nc.vector.max(out=m16[:, 0:8], in_=pkT2)
        nc.vector.match_replace(out=pkT2, in_to_replace=m16[:, 0:8],
                                in_values=pkT2, imm_value=-1e30)
        nc.vector.max(out=m16[:, 8:16], in_=pkT2)
        # regroup to one partition per head via PE transpose + DRAM bounce
        pT2 = psA2k("pT2")[:16, :128]
        nc.tensor.transpose(pT2, m16[:], ident[:])
        mTf = gp.tile([16, 128], F32)
        nc.vector.tensor_copy(mTf[:], pT2)
        mTd = dram.tile([16, 128], F32)
        w1 = nc.sync.dma_start(mTd[:], mTf[:])
        lvl3 = gp.tile([NH, 512], F32)
        r3 = nc.sync.dma_start(
            lvl3[:].rearrange("h (j r) -> h j r", j=32),
            mTd[:].rearrange("r (h j) -> h j r", h=NH))
        add_dep_helper(_raw(r3), _raw(w1), reason="lvl3 read after write")
        tops = gp.tile([NH, NCAND], F32)
        for rr in range(NCAND // 8):
            nc.vector.max(out=tops[:, ts(rr, 8)], in_=lvl3[:])
            if rr < NCAND // 8 - 1:
                nc.vector.match_replace(out=lvl3[:], in_to_replace=tops[:, ts(rr, 8)],
                                        in_values=lvl3[:], imm_value=-1e30)

        def decode_t(dst, src, n):
            t1 = gp.tile([NH, n], F32, tag="dec1")
            nc.vector.tensor_scalar_mul(t1[:], src, 8.0)
            t1i = gp.tile([NH, n], I32, tag="dec2")
            nc.vector.tensor_copy(t1i[:], t1[:])
            t1f = gp.tile([NH, n], F32, tag="dec3")
            nc.vector.tensor_copy(t1f[:], t1i[:])
            nc.vector.tensor_tensor(t1[:], t1[:], t1f[:], op=mybir.AluOpType.subtract)
            nc.vector.tensor_scalar_mul(dst, t1[:], 8192.0)

        cand_t = gp.tile([NH, NSLOT], F32)
        decode_t(cand_t[:, 0:NCAND], tops[:], NCAND)
        nc.vector.memset(cand_t[:, NCAND:NCAND + 1], 0.0)
        nc.vector.memset(cand_t[:, NCAND + 1:NSLOT], 4095.0)
        if DEBUG:
            nc.sync.dma_start(dbg["cand"], cand_t[:])

        pslt = psA2k("pslt")[:NSLOT, :NH]
        nc.tensor.transpose(pslt, cand_t[:], ident[:NH, :NH])
        ctf = gp.tile([NSLOT, NH], F32)
        nc.vector.tensor_copy(ctf[:], pslt)
        cti = gp.tile([NSLOT, NH], I32)
        nc.vector.tensor_copy(cti[:], ctf[:])

        # candidate-row gathers (hi|lo bf16 pairs) for all heads, issued
        # back-to-back so the software-DGE flights overlap; the deferred
        # local blocks keep PE busy while they land.
        xsels = []
        for h in range(NH):
            xsel = gbig.tile([NPAD, 2 * H], BF16, tag="xsel", bufs=4, name=f"xsel{h}")
            nc.gpsimd.indirect_dma_start(
                out=xsel[0:NSLOT, :], out_offset=None, in_=xhl_d,
                in_offset=bass.IndirectOffsetOnAxis(ap=cti[:, h:h + 1], axis=0))
            xsels.append(xsel)

        b0_def = A_DONE[0]
        for b in range(A_DONE[0], NB):
            if REST_B and b > b0_def and (b - b0_def) % 5 == 0:
                last = None
                for rr in range(2):
                    rdb = gw.tile([128, CW], BF16, tag="restb", bufs=2)
                    w = nc.sync.dma_start(rdb[:], xt_d[b % NCHUNK, 0, :, :])
                    if last is not None:
                        add_dep_helper(_raw(w), last, reason="rest chain B")
                    last = _raw(w)
                rest_gate[0] = last
            for h in range(NH):
                local_block(h, b)

        ne_all = gp.tile([NH, NSLOT], F32)
        qgTh = [None] * NH

        def prep_head(h):
            # exact re-projection of the candidate q rows (selection must
            # match the reference's fp32 norms bit-closely): host-split
            # bf16 hi+lo rows; q = xh@wh + xl@wh + xh@wl (xl@wl ~ 1e-6,
            # dropped). Slabs transposed by the DMA xbar (no PE cost).
            xhT = gbig.tile([128, KO, NSLOT], BF16, tag="xhT", bufs=2)
            xlT = gbig.tile([128, KO, NSLOT], BF16, tag="xlT", bufs=2)
            for half, dst in ((0, xhT), (1, xlT)):
                for kb in range(KO):
                    ptx = psum.tile([128, 1024], BF16, tag="A2k", bufs=2,
                                    name="ptx")[:, 0:NSLOT]
                    nc.tensor.transpose(
                        ptx, xsels[h][0:NSLOT, half * H + kb * 128:half * H + (kb + 1) * 128],
                        identb[:NSLOT, :NSLOT])
                    nc.vector.tensor_copy(dst[:, kb, :], ptx)
            pqc = psACC("pqc")[:, :NSLOT]
            for i, (w_, x_) in enumerate(((wqb, xhT), (wqb, xlT), (wlo, xhT))):
                for kb in range(KO):
                    nc.tensor.matmul(pqc, w_[:, kb, ts(h, D)], x_[:, kb, 0:NSLOT],
                                     start=(i == 0 and kb == 0),
                                     stop=(i == 2 and kb == KO - 1),
                                     skip_group_check=True)
            qcf = gw.tile([128, NSLOT], F32, tag="qcf")
            nc.vector.tensor_copy(qcf[:], pqc)
            qgTh[h] = gbig.tile([128, NSLOT], BF16, tag=f"qgT{h}", name=f"qgT{h}")
            nc.vector.tensor_copy(qgTh[h][:], qcf[:])
            sqc = gw.tile([128, NSLOT], F32, tag="sqc")
            nc.vector.tensor_tensor(sqc[:], qcf[:], qcf[:], op=mybir.AluOpType.mult)
            pne = psA2k("pne")[:1, :NSLOT]
            nc.tensor.matmul(pne, ones[:], sqc[:], start=True, stop=True)
            nerow = gw.tile([1, NSLOT], F32, tag="nerow")
            nc.vector.tensor_copy(nerow[:], pne)
            nc.scalar.dma_start(ne_all[h:h + 1, :], nerow[:])

        # software-pipelined: head h+1 preps while head h runs on PE
        PgTs = [None] * NH
        gcos = [None] * NH
        prep_head(0)
        PgTs[0] = global_scores(0)
        prep_head(1)
        gcos[0] = global_ctx(0, PgTs[0])
        PgTs[1] = global_scores(1)
        prep_head(2)
        gcos[1] = global_ctx(1, PgTs[1])
        PgTs[2] = global_scores(2)
        prep_head(3)
        if DEBUG:
            nc.sync.dma_start(dbg["ne"], ne_all[:])

        # threshold/selection chain (DVE; overlaps the PE work above)
        ne_work = gp.tile([NH, NSLOT], F32)
        nc.vector.tensor_copy(ne_work[:], ne_all[:])
        tops_e = gp.tile([NH, 64], F32)
        for rr in range(8):
            nc.vector.max(out=tops_e[:, ts(rr, 8)], in_=ne_work[:])
            if rr < 7:
                nc.vector.match_replace(out=ne_work[:], in_to_replace=tops_e[:, ts(rr, 8)],
                                        in_values=ne_work[:], imm_value=-1e30)
        theta = gp.tile([NH, 1], F32)
        nc.vector.tensor_copy(theta[:], tops_e[:, 61:62])

        # sel over the slots; specials (bos/eos) always selected
        sel = gp.tile([NH, NSLOT], F32)
        nc.vector.tensor_tensor(sel[:], ne_all[:], theta[:].to_broadcast([NH, NSLOT]),
                                op=mybir.AluOpType.is_ge)
        nc.vector.memset(sel[:, NCAND:NSLOT], 1.0)
        # scatter idx per slot: cand_t if selected else OOB (100000)
        sidx_f = gp.tile([NH, NSLOT], F32)
        nc.vector.tensor_scalar(sidx_f[:], sel[:], -1.0, scalar2=None,
                                op0=mybir.AluOpType.add)
        nc.vector.tensor_scalar_mul(sidx_f[:], sidx_f[:], -100000.0)
        nc.vector.tensor_tensor(sidx_f[:], sidx_f[:], cand_t[:], op=mybir.AluOpType.add)
        # fold head into the row index: row = token*NH + h (see scatter_head)
        nc.vector.tensor_scalar_mul(sidx_f[:], sidx_f[:], float(NH))
        hcol = gp.tile([NH, 1], F32)
        nc.gpsimd.iota(hcol[:], pattern=[[0, 1]], base=0, channel_multiplier=1,
                       allow_small_or_imprecise_dtypes=True)
        nc.vector.tensor_tensor(sidx_f[:], sidx_f[:],
                                hcol[:].to_broadcast([NH, NSLOT]),
                                op=mybir.AluOpType.add)
        p_ = psA2k("ptr")[:NSLOT, :NH]
        nc.tensor.transpose(p_, sidx_f[:], ident[:NH, :NH])
        sf1 = gp.tile([NSLOT, NH], F32)
        nc.vector.tensor_copy(sf1[:], p_)
        sidx_i = gp.tile([NSLOT, NH], I32)
        nc.vector.tensor_copy(sidx_i[:], sf1[:])
        if DEBUG:
            nc.sync.dma_start(dbg["sidx"], sidx_i[:])

        scatter_head(0, gcos[0])
        scatter_head(1, gcos[1])
        gcos[2] = global_ctx(2, PgTs[2])
        scatter_head(2, gcos[2])
        PgTs[3] = global_scores(3)
        gcos[3] = global_ctx(3, PgTs[3])
        scatter_head(3, gcos[3])

        gw.release()
        gbig.release()
        gp.release()
        wkv2.release()
        ab.release()
        psum.release()
        dram.release()
        res.release()
        const.release()

    nc.finalize()
    return nc


_NC_CACHE = None


def make_in_maps(inputs):
    import ml_dtypes
    BF = ml_dtypes.bfloat16
    hs = np.ascontiguousarray(np.asarray(inputs["hidden_states"], dtype=np.float32))
    Wq = np.ascontiguousarray(np.asarray(inputs["Wq"], dtype=np.float32))
    Wk = np.ascontiguousarray(np.asarray(inputs["Wk"], dtype=np.float32))
    Wv = np.ascontiguousarray(np.asarray(inputs["Wv"], dtype=np.float32))
    ident = np.eye(128, dtype=np.float32)
    # chunk-major [c, ko, p, t] layout (contiguous per-chunk slabs)
    xts_host = [
        np.ascontiguousarray(
            hs[n].T.astype(BF).reshape(16, 128, 8, 512).transpose(2, 0, 1, 3))
        for n in range(2)
    ]
    # hi|lo bf16 row pairs for the exact-norm gathers; the DMA-xbar
    # transpose lands hidden row r at (partition r%128, slab r//128),
    # matching the standard weight-slab convention.
    xhl_host = []
    for n in range(2):
        xh = hs[n].astype(BF)
        xl = (hs[n] - xh.astype(np.float32)).astype(BF)
        xhl_host.append(np.ascontiguousarray(
            np.concatenate([xh, xl], axis=1)))  # [T, 2H]
    Wq_bf = Wq.astype(BF)
    Wq_lo = (Wq - Wq_bf.astype(np.float32)).astype(BF)
    in_maps = []
    for c in range(8):
        n = c // 4
        h0 = (c % 4) * NH
        cols = slice(h0 * D, (h0 + NH) * D)
        in_maps.append({
            "xt": xts_host[n],
            "xhl": xhl_host[n],
            "wq": np.ascontiguousarray(Wq_bf[:, cols]),
            "wk": np.ascontiguousarray(Wk[:, cols].astype(BF)),
            "wv": np.ascontiguousarray(Wv[:, cols].astype(BF)),
            "wlo": np.ascontiguousarray(Wq_lo[:, cols]),
            "ident": ident,
            "identb": ident.astype(BF),
        })
    return in_maps


def kernel(**inputs):
    global _NC_CACHE
    if _NC_CACHE is None:
        _NC_CACHE = build_program()
    nc = _NC_CACHE
    in_maps = make_in_maps(inputs)
    res = run_bass_kernel_spmd(nc, in_maps, core_ids=list(range(8)))
    out = np.zeros((2, T, H), np.float32)
    for c in range(8):
        n = c // 4
        h0 = (c % 4) * NH
        out[n, :, h0 * D:(h0 + NH) * D] = res.results[c]["out"]
    return out


# revision 18
# speedup vs baseline: 1.0182x; 1.0182x over previous
"""Block-global self-attention Trainium2 kernel (SPMD over 8 NeuronCores).

Sharding: core c -> batch n = c//4, heads h0 = (c%4)*4 .. h0+3.
Each core receives xt = hidden[n].T (bf16, chunk-major) and wq/wk/wv =
W[:, cols] [2048,512] bf16, returns out [4096,512] (its head-column
stripe of batch n).

Per-core pipeline:
  P: bf16 projections (direct DMA of host-side bf16 xt/weights) -> qT/kT
     [d,t] + V2 (t-major, 64-row-shifted so local windows are two aligned
     full-K tiles). Approx q-norms land in a [128,NH,32] grid per chunk
     (DRAM bounce per chunk); the packed value (quantized norm + token id
     in low mantissa) transform + a DVE 32x32 stream-transpose into pkT
     [(h,j), pair, p] run incrementally under the chunk loop. Local
     blocks interleave with a 1-chunk lag; the last DEFER blocks are
     deferred to cover phase-B latency.
  A: local block attention; softmax without max-subtraction (|score|<8);
     probs kept unnormalized bf16, 1/denom fused into the final ACT copy.
  B: top-16 per pkT row (128-token classes) -> PE transpose + one DRAM
     bounce -> per-head pools [4,512] -> 9 max8/match_replace rounds ->
     top-72 candidates + bos/eos. Four indirect row gathers (hi|lo bf16
     pairs from host-packed xhl) issue back-to-back and overlap the
     deferred local blocks. Exact fp32 candidate q (hi/lo trick, wlo
     host-precomputed) via DMA-transposed slabs (no PE transposes);
     global attention transposed over all 74 slots; exact top-62
     threshold picks the final set; per-head scatters roll out as soon
     as selection + that head's ctx are ready.
"""
import os
import numpy as np

import concourse.bass as bass
import concourse.bacc as bacc
import concourse.mybir as mybir
from concourse.tile import TileContext, add_dep_helper
from concourse.bass_utils import run_bass_kernel_spmd

F32 = mybir.dt.float32
BF16 = mybir.dt.bfloat16
I32 = mybir.dt.int32

T = 4096
H = 2048
D = 128
NH = 4
KO = H // 128
NB = T // 128
CW = 512
NCHUNK = T // CW
NEG = -30.0
SCALE = float(1.0 / np.sqrt(128.0))
NCAND = 72
NSLOT = NCAND + 2
NPAD = 80          # gather/transpose row padding (xbar needs %16)
GEXP = 512 // NSLOT  # global score blocks per psum bank / exp call
DEBUG = bool(int(os.environ.get("KERNEL_DEBUG", "0")))
# PE rest: chained delay-DMAs per chunk boundary; breaks the sustained
# PE-activity streak that trips the P0 power-state downclock
REST = int(os.environ.get("KERNEL_REST", "1"))
DEFER = int(os.environ.get("KERNEL_DEFER", "16"))
REST_B = int(os.environ.get("KERNEL_REST_B", "0"))


def ts(i, sz):
    return slice(i * sz, (i + 1) * sz)


def _raw(inst):
    return inst.ins if hasattr(inst, "ins") else inst


def build_program():
    nc = bacc.Bacc("TRN2", target_bir_lowering=False, debug=False,
                   enable_asserts=True)
    # chunk-major xt layout: [c, ko, p, t] so each chunk DMA reads a
    # contiguous 512KB slab (sequential DRAM >> strided)
    xt_d = nc.dram_tensor("xt", (NCHUNK, KO, 128, CW), BF16,
                          kind="ExternalInput").ap()
    xhl_d = nc.dram_tensor("xhl", (T, 2 * H), BF16, kind="ExternalInput").ap()
    wq_d = nc.dram_tensor("wq", (H, NH * D), BF16, kind="ExternalInput").ap()
    wk_d = nc.dram_tensor("wk", (H, NH * D), BF16, kind="ExternalInput").ap()
    wv_d = nc.dram_tensor("wv", (H, NH * D), BF16, kind="ExternalInput").ap()
    wlo_d = nc.dram_tensor("wlo", (H, NH * D), BF16, kind="ExternalInput").ap()
    id_d = nc.dram_tensor("ident", (128, 128), F32, kind="ExternalInput").ap()
    idb_d = nc.dram_tensor("identb", (128, 128), BF16, kind="ExternalInput").ap()
    out_d = nc.dram_tensor("out", (T, NH * D), F32, kind="ExternalOutput").ap()
    dbg = {}
    if DEBUG:
        dbg["na"] = nc.dram_tensor("dbg_na", (128, NH, 32), F32, kind="ExternalOutput").ap()
        dbg["cand"] = nc.dram_tensor("dbg_cand", (NH, NSLOT), F32, kind="ExternalOutput").ap()
        dbg["ne"] = nc.dram_tensor("dbg_ne", (NH, NSLOT), F32, kind="ExternalOutput").ap()
        dbg["sidx"] = nc.dram_tensor("dbg_sidx", (NSLOT, NH), I32, kind="ExternalOutput").ap()

    with TileContext(nc) as tc:
        const = tc.alloc_tile_pool(name="const", bufs=1)
        res = tc.alloc_tile_pool(name="res", bufs=1)
        dram = tc.alloc_tile_pool(name="dram", bufs=1, space="DRAM")

        ident = const.tile([128, 128], F32)
        nc.sync.dma_start(ident[:], id_d)
        identb = const.tile([128, 128], BF16)
        nc.sync.dma_start(identb[:], idb_d)
        ones_b = const.tile([128, 1], BF16)
        nc.vector.memset(ones_b[:], 1.0)
        ones = const.tile([128, 1], F32)
        nc.vector.memset(ones[:], 1.0)
        iota_g = const.tile([128, NH, 32], F32)
        nc.gpsimd.iota(iota_g[:], pattern=[[0, NH], [1, 32]], base=0,
                       channel_multiplier=32, allow_small_or_imprecise_dtypes=True)
        kT = [res.tile([128, 64 + T + 64], BF16, tag=f"kT{h}", name=f"kT{h}") for h in range(NH)]
        V2 = res.tile([128, NB + 1, NH, D + 1], BF16, tag="V2")
        pkT = res.tile([128, NCHUNK // 2, 32], F32, tag="pkT")
        wqb = res.tile([128, KO, NH * D], BF16, tag="wqb")
        na_dram = dram.tile([NH, T], F32)

        # ---------------- pools ----------------
        psum = tc.alloc_tile_pool(name="psum", bufs=1, space="PSUM")
        ab = tc.alloc_tile_pool(name="ab", bufs=4)

        def psA2k(nm):   # 2KB f32 one-shot psums
            t = psum.tile([128, 512], F32, tag="A2k", bufs=2, name=nm)
            return t
        def psBLK(nm):   # per-block S + ctx combined
            t = psum.tile([128, 512], F32, tag="blk", bufs=2, name=nm)
            return t
        def psSG(nm):    # global score groups
            t = psum.tile([128, 512], F32, tag="psg", bufs=2, name=nm)
            return t
        def psACC(nm):   # held accumulators
            t = psum.tile([128, 512], F32, tag="ACC", bufs=2, name=nm)
            return t

        # ---------------- interleaved: local attention + global per head ----------------
        out_write_insts = []
        cur_co = [None]
        rest_gate = [None]

        def local_block(h, b):
            blk = psBLK("blk")
            # S^T halves: [tk(128), tq(128)]; half g covers window pos g*128..,
            # i.e. k tokens [b*128 - 64 + g*128, ...). kT is 64-padded.
            for g in range(2):
                seg = b + g
                mi = nc.tensor.matmul(blk[:, g * 128:(g + 1) * 128],
                                 kT[h][:, seg * 128:seg * 128 + 128],
                                 qT[h][:, ts(b, 128)], start=True, stop=True)
                if rest_gate[0] is not None:
                    add_dep_helper(_raw(mi), rest_gate[0], reason="rest gate")
                    rest_gate[0] = None
            PT = ab.tile([128, 256], BF16, tag="PT", name="PT", bufs=2)
            nc.scalar.activation(PT[:], blk[:, 0:256], mybir.ActivationFunctionType.Exp,
                                 scale=SCALE)
            pC = blk[:, 256:385]
            nc.tensor.matmul(pC, PT[:, 0:128], V2[:, b, h, :],
                             start=True, stop=False)
            nc.tensor.matmul(pC, PT[:, 128:256], V2[:, b + 1, h, :],
                             start=False, stop=True)
            rc = ab.tile([128, 1], F32, tag="rc", name="rc", bufs=8)
            nc.vector.reciprocal(rc[:], pC[:, 128:129])
            # all 4 heads of a block share one staging tile -> one 256KB
            # out write with 2KB rows (descriptor-rate-bound: 4x fewer DMAs)
            if h == 0:
                cur_co[0] = ab.tile([128, NH, D], F32, tag="co4", name="co4",
                                    bufs=2)
            nc.vector.tensor_scalar_mul(cur_co[0][:, h, :], pC[:, 0:D], rc[:])
            if h == NH - 1:
                w = nc.sync.dma_start(
                    out_d[ts(b, 128), :],
                    cur_co[0][:].rearrange("p h d -> p (h d)"))
                out_write_insts.append(_raw(w))

        def global_scores(h):
            # SgT blocks: psum [t(128), slot]; block jj covers tokens
            # jj*128-64 .. jj*128+63 (kT cols jj*128..+128, V2 block jj).
            # Pad tokens give exp(0)=1 but V2 values AND ones-col are 0
            # there, so they contribute nothing.
            PgT = gbig.tile([128, NB + 1, NSLOT], BF16, tag="PgT",
                            name=f"PgT{h}", bufs=2)
            jj = 0
            while jj <= NB:
                nb = min(GEXP, NB + 1 - jj)
                psg = psSG("psg")
                for gi in range(nb):
                    nc.tensor.matmul(psg[:, gi * NSLOT:(gi + 1) * NSLOT],
                                     kT[h][:, (jj + gi) * 128:(jj + gi + 1) * 128],
                                     qgTh[h][:], start=True, stop=True)
                nc.scalar.activation(
                    PgT[:, jj:jj + nb, :],
                    psg[:, 0:nb * NSLOT].rearrange("p (b s) -> p b s", b=nb),
                    mybir.ActivationFunctionType.Exp, scale=SCALE)
                jj += nb
            return PgT

        def global_ctx(h, PgT):
            pgc = psACC("pgc")[:NSLOT, :D + 1]
            for jj in range(NB + 1):
                nc.tensor.matmul(pgc, PgT[:, jj, :], V2[:, jj, h, :],
                                 start=(jj == 0), stop=(jj == NB),
                                 skip_group_check=True)
            rcg = gw.tile([NSLOT, 1], F32, tag="rcg", bufs=4)
            nc.vector.reciprocal(rcg[:], pgc[:, D:D + 1])
            gco = gw.tile([NSLOT, 128], F32, tag="gco", bufs=4)
            nc.vector.tensor_scalar_mul(gco[:], pgc[:, 0:D], rcg[:])
            return gco

        def scatter_head(h, gco):
            # out viewed as [T*NH, D] rows; sidx encodes token*NH + h so the
            # out AP keeps offset 0 (DynamicAP requirement)
            scat = nc.gpsimd.indirect_dma_start(
                out=out_d.rearrange("t (h d) -> (t h) d", h=NH),
                out_offset=bass.IndirectOffsetOnAxis(ap=sidx_i[:, h:h + 1], axis=0),
                in_=gco[:], in_offset=None,
                bounds_check=T * NH - 1, oob_is_err=False)
            for w in out_write_insts:
                add_dep_helper(_raw(scat), w, reason="scatter after local writes")


        A_DONE = [0]
        # ---------------- phase P ----------------
        wkv2 = tc.alloc_tile_pool(name="wkv2", bufs=1)
        wkv = tc.alloc_tile_pool(name="wkv", bufs=1)
        qT = [wkv2.tile([128, T], BF16, tag=f"qT{h}", name=f"qT{h}") for h in range(NH)]
        wkb = wkv.tile([128, KO, NH * D], BF16, tag="wkb")
        wvb = wkv.tile([128, KO, NH * D], BF16, tag="wvb")
        wb = {"q": wqb, "k": wkb, "v": wvb}

        with tc.tile_pool(name="pp", bufs=2) as pp, \
             tc.tile_pool(name="pp1", bufs=1) as pp1:

            xtb_tiles = {}

            def load_xtb(c):
                t = pp1.tile([128, KO, CW], BF16, tag="xtb", bufs=2)
                for kg in range(4):
                    nc.gpsimd.dma_start(
                        t[:, kg * 4:(kg + 1) * 4, :],
                        xt_d[c, kg * 4:(kg + 1) * 4, :, :].rearrange("ko p t -> p ko t"))
                return t

            # ramp order on the Pool queue: wq -> x chunk 0 -> wk -> wv
            # (queues serialize at the DMA arbiter, so issue in need-order)
            wrs = {nm: wd.rearrange("(ko p) m -> p ko m", p=128)
                   for nm, wd in (("q", wq_d), ("k", wk_d), ("v", wv_d))}
            wlor = wlo_d.rearrange("(ko p) m -> p ko m", p=128)
            nc.gpsimd.dma_start(wb["q"][:], wrs["q"][:])
            xtb_tiles[0] = load_xtb(0)
            for nm in ("k", "v"):
                nc.gpsimd.dma_start(wb[nm][:], wrs[nm][:])

            for h in range(NH):
                nc.vector.memset(kT[h][:, 0:64], 0.0)
                nc.vector.memset(kT[h][:, 64 + T:], 0.0)
            nc.vector.memset(V2[0:64, 0, :, :], 0.0)
            nc.vector.memset(V2[64:128, NB, :, :], 0.0)
            nc.vector.memset(V2[:, :, :, D:D + 1], 1.0)
            # pad rows contribute neither value nor denominator mass
            nc.vector.memset(V2[0:64, 0, :, D:D + 1], 0.0)
            nc.vector.memset(V2[64:128, NB, :, D:D + 1], 0.0)

            # packed-value transform scratch (slab-sliced per chunk)
            m0g = pp.tile([128, NH, 32], F32, tag="m0g", bufs=1)
            m1g = pp.tile([128, NH, 32], F32, tag="m1g", bufs=1)
            nagpg = pp.tile([128, NH, 32], F32, tag="nagpg", bufs=1)
            pkg = pp.tile([128, NH, 32], F32, tag="pkg", bufs=1)
            pkig = pp.tile([128, NH, 32], I32, tag="pkig", bufs=1)
            pkg2 = pkg[:].rearrange("p h j -> p (h j)")

            for c in range(NCHUNK):
                xtb = xtb_tiles.pop(c) if c in xtb_tiles else load_xtb(c)
                na_chunk_writes = []
                for h in range(NH):
                    for nm, dstT in (("q", qT[h]), ("k", kT[h])):
                        ps = psA2k("psqk")
                        for kb in range(KO):
                            mi = nc.tensor.matmul(ps[:], wb[nm][:, kb, ts(h, D)],
                                                  xtb[:, kb, :], start=(kb == 0),
                                                  stop=(kb == KO - 1))
                            if rest_gate[0] is not None:
                                add_dep_helper(_raw(mi), rest_gate[0],
                                               reason="PE rest gate")
                                rest_gate[0] = None
                        off = 64 if nm == "k" else 0
                        nc.vector.tensor_copy(dstT[:, off + c * CW:off + (c + 1) * CW], ps[:])
                        if nm == "q":
                            sq = pp.tile([128, CW], BF16, tag="sq", bufs=1)
                            nc.vector.tensor_tensor(sq[:], dstT[:, ts(c, CW)],
                                                    dstT[:, ts(c, CW)],
                                                    op=mybir.AluOpType.mult)
                            pn = psA2k("pn")[:1, :]
                            nc.tensor.matmul(pn, ones_b[:], sq[:],
                                             start=True, stop=True)
                            narow = pp.tile([1, CW], F32, tag="narow", bufs=1)
                            nc.vector.tensor_copy(narow[:], pn)
                            w = nc.sync.dma_start(na_dram[h:h + 1, ts(c, CW)], narow[:])
                            na_chunk_writes.append(_raw(w))
                for s in range(CW // 128):
                    sg = c * (CW // 128) + s
                    pv = psA2k("psv")
                    for kb in range(KO):
                        nc.tensor.matmul(pv[:], xtb[:, kb, ts(s, 128)],
                                         wb["v"][:, kb, :], start=(kb == 0),
                                         stop=(kb == KO - 1))
                    vt = pp.tile([128, NH * D], BF16, tag="vtmp", bufs=1)
                    nc.vector.tensor_copy(vt[:], pv[:])
                    nc.sync.dma_start(V2[64:128, sg, :, 0:D],
                                      vt[0:64, :].rearrange("p (h d) -> p h d", h=NH))
                    nc.sync.dma_start(V2[0:64, sg + 1, :, 0:D],
                                      vt[64:128, :].rearrange("p (h d) -> p h d", h=NH))
                # incremental norm grid + packed transform for this chunk
                # (tokens c*512.. live on grid partitions c*16..c*16+16)
                r = nc.sync.dma_start(
                    nagpg[ts(c, 16), :, :],
                    na_dram[:, ts(c, CW)].rearrange("h (p j) -> p h j", p=16))
                for w in na_chunk_writes:
                    add_dep_helper(_raw(r), w, reason="na slab read after writes")
                if c % 2 == 1:
                    # DVE partition offsets are quadrant-granular: transform
                    # the finished 32-partition chunk pair, then
                    # stream-transpose it into pkT rows (h*32+j)
                    t2 = c // 2
                    S = slice(t2 * 32, (t2 + 1) * 32)
                    nc.vector.tensor_scalar(m0g[S], iota_g[S], 0.0, scalar2=None,
                                            op0=mybir.AluOpType.is_equal)
                    nc.vector.tensor_scalar(m1g[S], iota_g[S], 4095.0, scalar2=None,
                                            op0=mybir.AluOpType.is_equal)
                    nc.vector.tensor_tensor(m0g[S], m0g[S], m1g[S], op=mybir.AluOpType.add)
                    nc.vector.tensor_tensor(m1g[S], nagpg[S], m0g[S], op=mybir.AluOpType.mult)
                    nc.vector.tensor_tensor(nagpg[S], nagpg[S], m1g[S], op=mybir.AluOpType.subtract)
                    nc.vector.tensor_scalar_mul(m0g[S], m0g[S], 1.0e6)
                    nc.vector.tensor_tensor(nagpg[S], nagpg[S], m0g[S], op=mybir.AluOpType.subtract)
                    nc.vector.tensor_scalar_mul(pkg[S], nagpg[S], 4.0)
                    nc.vector.tensor_copy(pkig[S], pkg[S])
                    nc.vector.tensor_copy(pkg[S], pkig[S])
                    nc.vector.tensor_scalar_mul(pkg[S], pkg[S], 0.125)
                    nc.vector.tensor_scalar_mul(m1g[S], iota_g[S], 2.0 ** -16)
                    nc.vector.tensor_tensor(pkg[S], pkg[S], m1g[S], op=mybir.AluOpType.add)
                    for jb in range(4):
                        nc.vector.transpose(pkT[ts(jb, 32), t2, :],
                                            pkg2[S, ts(jb, 32)])
                # interleave ready local-attention blocks (1-chunk lag);
                # hold back the last blocks to cover phase-B latency
                hi = min(4 * c - 2 + 1, NB - DEFER)
                for b in range(A_DONE[0], hi):
                    for h in range(NH):
                        local_block(h, b)
                A_DONE[0] = max(A_DONE[0], hi)
                if REST and c < NCHUNK - 1:
                    last = None
                    for rr in range(REST):
                        rd = dram.tile([128, 4, CW], BF16, tag="restd")
                        w = nc.gpsimd.dma_start(
                            rd[:], xt_d[c, 0:4, :, :].rearrange("ko p t -> p ko t"))
                        if last is not None:
                            add_dep_helper(_raw(w), last, reason="rest chain")
                        last = _raw(w)
                    rest_gate[0] = last

        wkv.release()

        # ---------------- phase B part 1: candidate top-72 funnel ----------------
        gp = tc.alloc_tile_pool(name="gp", bufs=1)
        gbig = tc.alloc_tile_pool(name="gbig", bufs=2)
        gw = tc.alloc_tile_pool(name="gw", bufs=2)
        # wq residual for the exact re-projection; only used in phase B so
        # loaded here (after wkv released its SBUF), overlapping the funnel
        wlo = gbig.tile([128, KO, NH * D], BF16, tag="wlo", bufs=1)
        nc.scalar.dma_start(wlo[:], wlor[:])

        # top-16 per pkT row (row = (h,j): 128 tokens {p*32+j}); top-72 of a
        # head has <=16 tokens in any such class w.h.p.
        pkT2 = pkT[:].rearrange("p t j -> p (t j)")
        m16 = gp.tile([128, 16], F32)
        nc.vector.max(out=m16[:, 0:8], in_=pkT2)
        nc.vector.match_replace(out=pkT2, in_to_replace=m16[:, 0:8],
                                in_values=pkT2, imm_value=-1e30)
        nc.vector.max(out=m16[:, 8:16], in_=pkT2)
        # regroup to one partition per head via PE transpose + DRAM bounce
        pT2 = psA2k("pT2")[:16, :128]
        nc.tensor.transpose(pT2, m16[:], ident[:])
        mTf = gp.tile([16, 128], F32)
        nc.vector.tensor_copy(mTf[:], pT2)
        mTd = dram.tile([16, 128], F32)
        w1 = nc.sync.dma_start(mTd[:], mTf[:])
        lvl3 = gp.tile([NH, 512], F32)
        r3 = nc.sync.dma_start(
            lvl3[:].rearrange("h (j r) -> h j r", j=32),
            mTd[:].rearrange("r (h j) -> h j r", h=NH))
        add_dep_helper(_raw(r3), _raw(w1), reason="lvl3 read after write")
        tops = gp.tile([NH, NCAND], F32)
        for rr in range(NCAND // 8):
            nc.vector.max(out=tops[:, ts(rr, 8)], in_=lvl3[:])
            if rr < NCAND // 8 - 1:
                nc.vector.match_replace(out=lvl3[:], in_to_replace=tops[:, ts(rr, 8)],
                                        in_values=lvl3[:], imm_value=-1e30)

        def decode_t(dst, src, n):
            t1 = gp.tile([NH, n], F32, tag="dec1")
            nc.vector.tensor_scalar_mul(t1[:], src, 8.0)
            t1i = gp.tile([NH, n], I32, tag="dec2")
            nc.vector.tensor_copy(t1i[:], t1[:])
            t1f = gp.tile([NH, n], F32, tag="dec3")
            nc.vector.tensor_copy(t1f[:], t1i[:])
            nc.vector.tensor_tensor(t1[:], t1[:], t1f[:], op=mybir.AluOpType.subtract)
            nc.vector.tensor_scalar_mul(dst, t1[:], 8192.0)

        cand_t = gp.tile([NH, NSLOT], F32)
        decode_t(cand_t[:, 0:NCAND], tops[:], NCAND)
        nc.vector.memset(cand_t[:, NCAND:NCAND + 1], 0.0)
        nc.vector.memset(cand_t[:, NCAND + 1:NSLOT], 4095.0)
        if DEBUG:
            nc.sync.dma_start(dbg["cand"], cand_t[:])

        pslt = psA2k("pslt")[:NSLOT, :NH]
        nc.tensor.transpose(pslt, cand_t[:], ident[:NH, :NH])
        ctf = gp.tile([NSLOT, NH], F32)
        nc.vector.tensor_copy(ctf[:], pslt)
        cti = gp.tile([NSLOT, NH], I32)
        nc.vector.tensor_copy(cti[:], ctf[:])

        # candidate-row gathers (hi|lo bf16 pairs) for all heads, issued
        # back-to-back so the software-DGE flights overlap; the deferred
        # local blocks keep PE busy while they land.
        xsels = []
        for h in range(NH):
            xsel = gbig.tile([NPAD, 2 * H], BF16, tag="xsel", bufs=4, name=f"xsel{h}")
            nc.gpsimd.indirect_dma_start(
                out=xsel[0:NSLOT, :], out_offset=None, in_=xhl_d,
                in_offset=bass.IndirectOffsetOnAxis(ap=cti[:, h:h + 1], axis=0))
            xsels.append(xsel)

        b0_def = A_DONE[0]
        for b in range(A_DONE[0], NB):
            if REST_B and b > b0_def and (b - b0_def) % 5 == 0:
                last = None
                for rr in range(2):
                    rdb = gw.tile([128, CW], BF16, tag="restb", bufs=2)
                    w = nc.sync.dma_start(rdb[:], xt_d[b % NCHUNK, 0, :, :])
                    if last is not None:
                        add_dep_helper(_raw(w), last, reason="rest chain B")
                    last = _raw(w)
                rest_gate[0] = last
            for h in range(NH):
                local_block(h, b)

        ne_all = gp.tile([NH, NSLOT], F32)
        qgTh = [None] * NH

        def prep_head(h):
            # exact re-projection of the candidate q rows (selection must
            # match the reference's fp32 norms bit-closely): host-split
            # bf16 hi+lo rows; q = xh@wh + xl@wh + xh@wl (xl@wl ~ 1e-6,
            # dropped). Slabs transposed by the DMA xbar (no PE cost).
            xhT = gbig.tile([128, KO, NSLOT], BF16, tag="xhT", bufs=2)
            xlT = gbig.tile([128, KO, NSLOT], BF16, tag="xlT", bufs=2)
            for half, dst in ((0, xhT), (1, xlT)):
                for kb in range(KO):
                    ptx = psum.tile([128, 1024], BF16, tag="A2k", bufs=2,
                                    name="ptx")[:, 0:NSLOT]
                    nc.tensor.transpose(
                        ptx, xsels[h][0:NSLOT, half * H + kb * 128:half * H + (kb + 1) * 128],
                        identb[:NSLOT, :NSLOT])
                    nc.vector.tensor_copy(dst[:, kb, :], ptx)
            pqc = psACC("pqc")[:, :NSLOT]
            for i, (w_, x_) in enumerate(((wqb, xhT), (wqb, xlT), (wlo, xhT))):
                for kb in range(KO):
                    nc.tensor.matmul(pqc, w_[:, kb, ts(h, D)], x_[:, kb, 0:NSLOT],
                                     start=(i == 0 and kb == 0),
                                     stop=(i == 2 and kb == KO - 1),
                                     skip_group_check=True)
            qcf = gw.tile([128, NSLOT], F32, tag="qcf")
            nc.vector.tensor_copy(qcf[:], pqc)
            qgTh[h] = gbig.tile([128, NSLOT], BF16, tag=f"qgT{h}", name=f"qgT{h}")
            nc.vector.tensor_copy(qgTh[h][:], qcf[:])
            sqc = gw.tile([128, NSLOT], F32, tag="sqc")
            nc.vector.tensor_tensor(sqc[:], qcf[:], qcf[:], op=mybir.AluOpType.mult)
            pne = psA2k("pne")[:1, :NSLOT]
            nc.tensor.matmul(pne, ones[:], sqc[:], start=True, stop=True)
            nerow = gw.tile([1, NSLOT], F32, tag="nerow")
            nc.vector.tensor_copy(nerow[:], pne)
            nc.scalar.dma_start(ne_all[h:h + 1, :], nerow[:])

        # software-pipelined: head h+1 preps while head h runs on PE
        PgTs = [None] * NH
        gcos = [None] * NH
        prep_head(0)
        PgTs[0] = global_scores(0)
        prep_head(1)
        gcos[0] = global_ctx(0, PgTs[0])
        PgTs[1] = global_scores(1)
        prep_head(2)
        gcos[1] = global_ctx(1, PgTs[1])
        PgTs[2] = global_scores(2)
        prep_head(3)
        if DEBUG:
            nc.sync.dma_start(dbg["ne"], ne_all[:])

        # threshold/selection chain (DVE; overlaps the PE work above)
        ne_work = gp.tile([NH, NSLOT], F32)
        nc.vector.tensor_copy(ne_work[:], ne_all[:])
        tops_e = gp.tile([NH, 64], F32)
        for rr in range(8):
            nc.vector.max(out=tops_e[:, ts(rr, 8)], in_=ne_work[:])
            if rr < 7:
                nc.vector.match_replace(out=ne_work[:], in_to_replace=tops_e[:, ts(rr, 8)],
                                        in_values=ne_work[:], imm_value=-1e30)
        theta = gp.tile([NH, 1], F32)
        nc.vector.tensor_copy(theta[:], tops_e[:, 61:62])

        # sel over the slots; specials (bos/eos) always selected
        sel = gp.tile([NH, NSLOT], F32)
        nc.vector.tensor_tensor(sel[:], ne_all[:], theta[:].to_broadcast([NH, NSLOT]),
                                op=mybir.AluOpType.is_ge)
        nc.vector.memset(sel[:, NCAND:NSLOT], 1.0)
        # scatter idx per slot: cand_t if selected else OOB (100000)
        sidx_f = gp.tile([NH, NSLOT], F32)
        nc.vector.tensor_scalar(sidx_f[:], sel[:], -1.0, scalar2=None,
                                op0=mybir.AluOpType.add)
        nc.vector.tensor_scalar_mul(sidx_f[:], sidx_f[:], -100000.0)
        nc.vector.tensor_tensor(sidx_f[:], sidx_f[:], cand_t[:], op=mybir.AluOpType.add)
        # fold head into the row index: row = token*NH + h (see scatter_head)
        nc.vector.tensor_scalar_mul(sidx_f[:], sidx_f[:], float(NH))
        hcol = gp.tile([NH, 1], F32)
        nc.gpsimd.iota(hcol[:], pattern=[[0, 1]], base=0, channel_multiplier=1,
                       allow_small_or_imprecise_dtypes=True)
        nc.vector.tensor_tensor(sidx_f[:], sidx_f[:],
                                hcol[:].to_broadcast([NH, NSLOT]),
                                op=mybir.AluOpType.add)
        p_ = psA2k("ptr")[:NSLOT, :NH]
        nc.tensor.transpose(p_, sidx_f[:], ident[:NH, :NH])
        sf1 = gp.tile([NSLOT, NH], F32)
        nc.vector.tensor_copy(sf1[:], p_)
        sidx_i = gp.tile([NSLOT, NH], I32)
        nc.vector.tensor_copy(sidx_i[:], sf1[:])
        if DEBUG:
            nc.sync.dma_start(dbg["sidx"], sidx_i[:])

        scatter_head(0, gcos[0])
        scatter_head(1, gcos[1])
        gcos[2] = global_ctx(2, PgTs[2])
        scatter_head(2, gcos[2])
        PgTs[3] = global_scores(3)
        gcos[3] = global_ctx(3, PgTs[3])
        scatter_head(3, gcos[3])

        gw.release()
        gbig.release()
        gp.release()
        wkv2.release()
        ab.release()
        psum.release()
        dram.release()
        res.release()
        const.release()

    nc.finalize()
    return nc


_NC_CACHE = None


def make_in_maps(inputs):
    import ml_dtypes
    BF = ml_dtypes.bfloat16
    hs = np.ascontiguousarray(np.asarray(inputs["hidden_states"], dtype=np.float32))
    Wq = np.ascontiguousarray(np.asarray(inputs["Wq"], dtype=np.float32))
    Wk = np.ascontiguousarray(np.asarray(inputs["Wk"], dtype=np.float32))
    Wv = np.ascontiguousarray(np.asarray(inputs["Wv"], dtype=np.float32))
    ident = np.eye(128, dtype=np.float32)
    # chunk-major [c, ko, p, t] layout (contiguous per-chunk slabs)
    xts_host = [
        np.ascontiguousarray(
            hs[n].T.astype(BF).reshape(16, 128, 8, 512).transpose(2, 0, 1, 3))
        for n in range(2)
    ]
    # hi|lo bf16 row pairs for the exact-norm gathers; the DMA-xbar
    # transpose lands hidden row r at (partition r%128, slab r//128),
    # matching the standard weight-slab convention.
    xhl_host = []
    for n in range(2):
        xh = hs[n].astype(BF)
        xl = (hs[n] - xh.astype(np.float32)).astype(BF)
        xhl_host.append(np.ascontiguousarray(
            np.concatenate([xh, xl], axis=1)))  # [T, 2H]
    Wq_bf = Wq.astype(BF)
    Wq_lo = (Wq - Wq_bf.astype(np.float32)).astype(BF)
    in_maps = []
    for c in range(8):
        n = c // 4
        h0 = (c % 4) * NH
        cols = slice(h0 * D, (h0 + NH) * D)
        in_maps.append({
            "xt": xts_host[n],
            "xhl": xhl_host[n],
            "wq": np.ascontiguousarray(Wq_bf[:, cols]),
            "wk": np.ascontiguousarray(Wk[:, cols].astype(BF)),
            "wv": np.ascontiguousarray(Wv[:, cols].astype(BF)),
            "wlo": np.ascontiguousarray(Wq_lo[:, cols]),
            "ident": ident,
            "identb": ident.astype(BF),
        })
    return in_maps


def kernel(**inputs):
    global _NC_CACHE
    if _NC_CACHE is None:
        _NC_CACHE = build_program()
    nc = _NC_CACHE
    in_maps = make_in_maps(inputs)
    res = run_bass_kernel_spmd(nc, in_maps, core_ids=list(range(8)))
    out = np.zeros((2, T, H), np.float32)
    for c in range(8):
        n = c // 4
        h0 = (c % 4) * NH
        out[n, :, h0 * D:(h0 + NH) * D] = res.results[c]["out"]
    return out


# revision 19
# speedup vs baseline: 1.0470x; 1.0282x over previous
"""Block-global self-attention Trainium2 kernel (SPMD over 8 NeuronCores).

Sharding: core c -> batch n = c//4, heads h0 = (c%4)*4 .. h0+3.
Each core receives xt = hidden[n].T (bf16, chunk-major) and wq/wk/wv =
W[:, cols] [2048,512] bf16, returns out [4096,512] (its head-column
stripe of batch n).

Per-core pipeline:
  P: bf16 projections (direct DMA of host-side bf16 xt/weights) -> qT/kT
     [d,t] + V2 (t-major, 64-row-shifted so local windows are two aligned
     full-K tiles). Approx q-norms land in a [128,NH,32] grid per chunk
     (DRAM bounce per chunk); the packed value (quantized norm + token id
     in low mantissa) transform + a DVE 32x32 stream-transpose into pkT
     [(h,j), pair, p] run incrementally under the chunk loop. Local
     blocks interleave with a 1-chunk lag; the last DEFER blocks are
     deferred to cover phase-B latency.
  A: local block attention; softmax without max-subtraction (|score|<8);
     probs kept unnormalized bf16, 1/denom fused into the final ACT copy.
  B: top-16 per pkT row (128-token classes) -> PE transpose + one DRAM
     bounce -> per-head pools [4,512] -> 9 max8/match_replace rounds ->
     top-72 candidates + bos/eos. Four indirect row gathers (hi|lo bf16
     pairs from host-packed xhl) issue back-to-back and overlap the
     deferred local blocks. Exact fp32 candidate q (hi/lo trick, wlo
     host-precomputed) via DMA-transposed slabs (no PE transposes);
     global attention transposed over all 74 slots; exact top-62
     threshold picks the final set; per-head scatters roll out as soon
     as selection + that head's ctx are ready.
"""
import os
import numpy as np

import concourse.bass as bass
import concourse.bacc as bacc
import concourse.mybir as mybir
from concourse.tile import TileContext, add_dep_helper
from concourse.bass_utils import run_bass_kernel_spmd

F32 = mybir.dt.float32
BF16 = mybir.dt.bfloat16
I32 = mybir.dt.int32

T = 4096
H = 2048
D = 128
NH = 4
KO = H // 128
NB = T // 128
CW = 512
NCHUNK = T // CW
NEG = -30.0
SCALE = float(1.0 / np.sqrt(128.0))
NCAND = 72
NSLOT = NCAND + 2
NPAD = 80          # gather/transpose row padding (xbar needs %16)
GEXP = 512 // NSLOT  # global score blocks per psum bank / exp call
DEBUG = bool(int(os.environ.get("KERNEL_DEBUG", "0")))
# PE rest: chained delay-DMAs per chunk boundary; breaks the sustained
# PE-activity streak that trips the P0 power-state downclock
REST = int(os.environ.get("KERNEL_REST", "1"))
DEFER = int(os.environ.get("KERNEL_DEFER", "16"))
REST_B = int(os.environ.get("KERNEL_REST_B", "0"))


def ts(i, sz):
    return slice(i * sz, (i + 1) * sz)


def _raw(inst):
    return inst.ins if hasattr(inst, "ins") else inst


def build_program():
    nc = bacc.Bacc("TRN2", target_bir_lowering=False, debug=False,
                   enable_asserts=True)
    # chunk-major xt layout: [c, ko, p, t] so each chunk DMA reads a
    # contiguous 512KB slab (sequential DRAM >> strided)
    xt_d = nc.dram_tensor("xt", (NCHUNK, KO, 128, CW), BF16,
                          kind="ExternalInput").ap()
    xhl_d = nc.dram_tensor("xhl", (T, 2 * H), BF16, kind="ExternalInput").ap()
    wq_d = nc.dram_tensor("wq", (H, NH * D), BF16, kind="ExternalInput").ap()
    wk_d = nc.dram_tensor("wk", (H, NH * D), BF16, kind="ExternalInput").ap()
    wv_d = nc.dram_tensor("wv", (H, NH * D), BF16, kind="ExternalInput").ap()
    wlo_d = nc.dram_tensor("wlo", (H, NH * D), BF16, kind="ExternalInput").ap()
    id_d = nc.dram_tensor("ident", (128, 128), F32, kind="ExternalInput").ap()
    idb_d = nc.dram_tensor("identb", (128, 128), BF16, kind="ExternalInput").ap()
    out_d = nc.dram_tensor("out", (T, NH * D), F32, kind="ExternalOutput").ap()
    dbg = {}
    if DEBUG:
        dbg["na"] = nc.dram_tensor("dbg_na", (128, NH, 32), F32, kind="ExternalOutput").ap()
        dbg["cand"] = nc.dram_tensor("dbg_cand", (NH, NSLOT), F32, kind="ExternalOutput").ap()
        dbg["ne"] = nc.dram_tensor("dbg_ne", (NH, NSLOT), F32, kind="ExternalOutput").ap()
        dbg["sidx"] = nc.dram_tensor("dbg_sidx", (NSLOT, NH), I32, kind="ExternalOutput").ap()

    with TileContext(nc) as tc:
        const = tc.alloc_tile_pool(name="const", bufs=1)
        res = tc.alloc_tile_pool(name="res", bufs=1)
        dram = tc.alloc_tile_pool(name="dram", bufs=1, space="DRAM")

        ident = const.tile([128, 128], F32)
        nc.sync.dma_start(ident[:], id_d)
        identb = const.tile([128, 128], BF16)
        nc.sync.dma_start(identb[:], idb_d)
        ones_b = const.tile([128, 1], BF16)
        nc.vector.memset(ones_b[:], 1.0)
        ones = const.tile([128, 1], F32)
        nc.vector.memset(ones[:], 1.0)
        iota_g = const.tile([128, NH, 32], F32)
        nc.gpsimd.iota(iota_g[:], pattern=[[0, NH], [1, 32]], base=0,
                       channel_multiplier=32, allow_small_or_imprecise_dtypes=True)
        kT = [res.tile([128, 64 + T + 64], BF16, tag=f"kT{h}", name=f"kT{h}") for h in range(NH)]
        V2 = res.tile([128, NB + 1, NH, D + 1], BF16, tag="V2")
        pkT = res.tile([128, NCHUNK // 2, 32], F32, tag="pkT")
        wqb = res.tile([128, KO, NH * D], BF16, tag="wqb")
        na_dram = dram.tile([NH, T], F32)

        # ---------------- pools ----------------
        psum = tc.alloc_tile_pool(name="psum", bufs=1, space="PSUM")
        ab = tc.alloc_tile_pool(name="ab", bufs=4)

        def psA2k(nm):   # 2KB f32 one-shot psums
            t = psum.tile([128, 512], F32, tag="A2k", bufs=2, name=nm)
            return t
        def psBLK(nm):   # per-block S + ctx combined
            t = psum.tile([128, 512], F32, tag="blk", bufs=2, name=nm)
            return t
        def psSG(nm):    # global score groups
            t = psum.tile([128, 512], F32, tag="psg", bufs=2, name=nm)
            return t
        def psACC(nm):   # held accumulators
            t = psum.tile([128, 512], F32, tag="ACC", bufs=2, name=nm)
            return t

        # ---------------- interleaved: local attention + global per head ----------------
        out_write_insts = []
        cur_co = [None]
        rest_gate = [None]

        def local_block(h, b):
            blk = psBLK("blk")
            # S^T halves: [tk(128), tq(128)]; half g covers window pos g*128..,
            # i.e. k tokens [b*128 - 64 + g*128, ...). kT is 64-padded.
            for g in range(2):
                seg = b + g
                mi = nc.tensor.matmul(blk[:, g * 128:(g + 1) * 128],
                                 kT[h][:, seg * 128:seg * 128 + 128],
                                 qT[h][:, ts(b, 128)], start=True, stop=True)
                if rest_gate[0] is not None:
                    add_dep_helper(_raw(mi), rest_gate[0], reason="rest gate")
                    rest_gate[0] = None
            PT = ab.tile([128, 256], BF16, tag="PT", name="PT", bufs=2)
            nc.scalar.activation(PT[:], blk[:, 0:256], mybir.ActivationFunctionType.Exp,
                                 scale=SCALE)
            pC = blk[:, 256:385]
            nc.tensor.matmul(pC, PT[:, 0:128], V2[:, b, h, :],
                             start=True, stop=False)
            nc.tensor.matmul(pC, PT[:, 128:256], V2[:, b + 1, h, :],
                             start=False, stop=True)
            rc = ab.tile([128, 1], F32, tag="rc", name="rc", bufs=8)
            nc.vector.reciprocal(rc[:], pC[:, 128:129])
            # all 4 heads of a block share one staging tile -> one 256KB
            # out write with 2KB rows (descriptor-rate-bound: 4x fewer DMAs)
            if h == 0:
                cur_co[0] = ab.tile([128, NH, D], F32, tag="co4", name="co4",
                                    bufs=2)
            nc.scalar.activation(cur_co[0][:, h, :], pC[:, 0:D],
                                 mybir.ActivationFunctionType.Copy, scale=rc[:])
            if h == NH - 1:
                w = nc.sync.dma_start(
                    out_d[ts(b, 128), :],
                    cur_co[0][:].rearrange("p h d -> p (h d)"))
                out_write_insts.append(_raw(w))

        def global_scores(h):
            # SgT blocks: psum [t(128), slot]; block jj covers tokens
            # jj*128-64 .. jj*128+63 (kT cols jj*128..+128, V2 block jj).
            # Pad tokens give exp(0)=1 but V2 values AND ones-col are 0
            # there, so they contribute nothing.
            PgT = gbig.tile([128, NB + 1, NSLOT], BF16, tag="PgT",
                            name=f"PgT{h}", bufs=2)
            jj = 0
            while jj <= NB:
                nb = min(GEXP, NB + 1 - jj)
                psg = psSG("psg")
                for gi in range(nb):
                    nc.tensor.matmul(psg[:, gi * NSLOT:(gi + 1) * NSLOT],
                                     kT[h][:, (jj + gi) * 128:(jj + gi + 1) * 128],
                                     qgTh[h][:], start=True, stop=True)
                nc.scalar.activation(
                    PgT[:, jj:jj + nb, :],
                    psg[:, 0:nb * NSLOT].rearrange("p (b s) -> p b s", b=nb),
                    mybir.ActivationFunctionType.Exp, scale=SCALE)
                jj += nb
            return PgT

        def global_ctx(h, PgT):
            pgc = psACC("pgc")[:NSLOT, :D + 1]
            for jj in range(NB + 1):
                nc.tensor.matmul(pgc, PgT[:, jj, :], V2[:, jj, h, :],
                                 start=(jj == 0), stop=(jj == NB),
                                 skip_group_check=True)
            rcg = gw.tile([NSLOT, 1], F32, tag="rcg", bufs=4)
            nc.vector.reciprocal(rcg[:], pgc[:, D:D + 1])
            gco = gw.tile([NSLOT, 128], F32, tag="gco", bufs=4)
            nc.scalar.activation(gco[:], pgc[:, 0:D],
                                 mybir.ActivationFunctionType.Copy, scale=rcg[:])
            return gco

        def scatter_head(h, gco):
            # out viewed as [T*NH, D] rows; sidx encodes token*NH + h so the
            # out AP keeps offset 0 (DynamicAP requirement)
            scat = nc.gpsimd.indirect_dma_start(
                out=out_d.rearrange("t (h d) -> (t h) d", h=NH),
                out_offset=bass.IndirectOffsetOnAxis(ap=sidx_i[:, h:h + 1], axis=0),
                in_=gco[:], in_offset=None,
                bounds_check=T * NH - 1, oob_is_err=False)
            for w in out_write_insts:
                add_dep_helper(_raw(scat), w, reason="scatter after local writes")


        A_DONE = [0]
        # ---------------- phase P ----------------
        wkv2 = tc.alloc_tile_pool(name="wkv2", bufs=1)
        wkv = tc.alloc_tile_pool(name="wkv", bufs=1)
        qT = [wkv2.tile([128, T], BF16, tag=f"qT{h}", name=f"qT{h}") for h in range(NH)]
        wkb = wkv.tile([128, KO, NH * D], BF16, tag="wkb")
        wvb = wkv.tile([128, KO, NH * D], BF16, tag="wvb")
        wb = {"q": wqb, "k": wkb, "v": wvb}

        with tc.tile_pool(name="pp", bufs=2) as pp, \
             tc.tile_pool(name="pp1", bufs=1) as pp1:

            xtb_tiles = {}

            def load_xtb(c):
                t = pp1.tile([128, KO, CW], BF16, tag="xtb", bufs=2)
                for kg in range(4):
                    nc.gpsimd.dma_start(
                        t[:, kg * 4:(kg + 1) * 4, :],
                        xt_d[c, kg * 4:(kg + 1) * 4, :, :].rearrange("ko p t -> p ko t"))
                return t

            # ramp order on the Pool queue: wq -> x chunk 0 -> wk -> wv
            # (queues serialize at the DMA arbiter, so issue in need-order)
            wrs = {nm: wd.rearrange("(ko p) m -> p ko m", p=128)
                   for nm, wd in (("q", wq_d), ("k", wk_d), ("v", wv_d))}
            wlor = wlo_d.rearrange("(ko p) m -> p ko m", p=128)
            nc.gpsimd.dma_start(wb["q"][:], wrs["q"][:])
            xtb_tiles[0] = load_xtb(0)
            for nm in ("k", "v"):
                nc.gpsimd.dma_start(wb[nm][:], wrs[nm][:])

            for h in range(NH):
                nc.vector.memset(kT[h][:, 0:64], 0.0)
                nc.vector.memset(kT[h][:, 64 + T:], 0.0)
            nc.vector.memset(V2[0:64, 0, :, :], 0.0)
            nc.vector.memset(V2[64:128, NB, :, :], 0.0)
            nc.vector.memset(V2[:, :, :, D:D + 1], 1.0)
            # pad rows contribute neither value nor denominator mass
            nc.vector.memset(V2[0:64, 0, :, D:D + 1], 0.0)
            nc.vector.memset(V2[64:128, NB, :, D:D + 1], 0.0)

            # packed-value transform scratch (slab-sliced per chunk)
            m0g = pp.tile([128, NH, 32], F32, tag="m0g", bufs=1)
            m1g = pp.tile([128, NH, 32], F32, tag="m1g", bufs=1)
            nagpg = pp.tile([128, NH, 32], F32, tag="nagpg", bufs=1)
            pkg = pp.tile([128, NH, 32], F32, tag="pkg", bufs=1)
            pkig = pp.tile([128, NH, 32], I32, tag="pkig", bufs=1)
            pkg2 = pkg[:].rearrange("p h j -> p (h j)")

            for c in range(NCHUNK):
                xtb = xtb_tiles.pop(c) if c in xtb_tiles else load_xtb(c)
                na_chunk_writes = []
                for h in range(NH):
                    for nm, dstT in (("q", qT[h]), ("k", kT[h])):
                        ps = psA2k("psqk")
                        for kb in range(KO):
                            mi = nc.tensor.matmul(ps[:], wb[nm][:, kb, ts(h, D)],
                                                  xtb[:, kb, :], start=(kb == 0),
                                                  stop=(kb == KO - 1))
                            if rest_gate[0] is not None:
                                add_dep_helper(_raw(mi), rest_gate[0],
                                               reason="PE rest gate")
                                rest_gate[0] = None
                        off = 64 if nm == "k" else 0
                        nc.vector.tensor_copy(dstT[:, off + c * CW:off + (c + 1) * CW], ps[:])
                        if nm == "q":
                            sq = pp.tile([128, CW], BF16, tag="sq", bufs=1)
                            nc.vector.tensor_tensor(sq[:], dstT[:, ts(c, CW)],
                                                    dstT[:, ts(c, CW)],
                                                    op=mybir.AluOpType.mult)
                            pn = psA2k("pn")[:1, :]
                            nc.tensor.matmul(pn, ones_b[:], sq[:],
                                             start=True, stop=True)
                            narow = pp.tile([1, CW], F32, tag="narow", bufs=1)
                            nc.vector.tensor_copy(narow[:], pn)
                            w = nc.sync.dma_start(na_dram[h:h + 1, ts(c, CW)], narow[:])
                            na_chunk_writes.append(_raw(w))
                for s in range(CW // 128):
                    sg = c * (CW // 128) + s
                    pv = psA2k("psv")
                    for kb in range(KO):
                        nc.tensor.matmul(pv[:], xtb[:, kb, ts(s, 128)],
                                         wb["v"][:, kb, :], start=(kb == 0),
                                         stop=(kb == KO - 1))
                    vt = pp.tile([128, NH * D], BF16, tag="vtmp", bufs=1)
                    nc.vector.tensor_copy(vt[:], pv[:])
                    nc.sync.dma_start(V2[64:128, sg, :, 0:D],
                                      vt[0:64, :].rearrange("p (h d) -> p h d", h=NH))
                    nc.sync.dma_start(V2[0:64, sg + 1, :, 0:D],
                                      vt[64:128, :].rearrange("p (h d) -> p h d", h=NH))
                # incremental norm grid + packed transform for this chunk
                # (tokens c*512.. live on grid partitions c*16..c*16+16)
                r = nc.sync.dma_start(
                    nagpg[ts(c, 16), :, :],
                    na_dram[:, ts(c, CW)].rearrange("h (p j) -> p h j", p=16))
                for w in na_chunk_writes:
                    add_dep_helper(_raw(r), w, reason="na slab read after writes")
                if c % 2 == 1:
                    # DVE partition offsets are quadrant-granular: transform
                    # the finished 32-partition chunk pair, then
                    # stream-transpose it into pkT rows (h*32+j)
                    t2 = c // 2
                    S = slice(t2 * 32, (t2 + 1) * 32)
                    nc.vector.tensor_scalar(m0g[S], iota_g[S], 0.0, scalar2=None,
                                            op0=mybir.AluOpType.is_equal)
                    nc.vector.tensor_scalar(m1g[S], iota_g[S], 4095.0, scalar2=None,
                                            op0=mybir.AluOpType.is_equal)
                    nc.vector.tensor_tensor(m0g[S], m0g[S], m1g[S], op=mybir.AluOpType.add)
                    nc.vector.tensor_tensor(m1g[S], nagpg[S], m0g[S], op=mybir.AluOpType.mult)
                    nc.vector.tensor_tensor(nagpg[S], nagpg[S], m1g[S], op=mybir.AluOpType.subtract)
                    nc.vector.tensor_scalar_mul(m0g[S], m0g[S], 1.0e6)
                    nc.vector.tensor_tensor(nagpg[S], nagpg[S], m0g[S], op=mybir.AluOpType.subtract)
                    nc.vector.tensor_scalar_mul(pkg[S], nagpg[S], 4.0)
                    nc.vector.tensor_copy(pkig[S], pkg[S])
                    nc.vector.tensor_copy(pkg[S], pkig[S])
                    nc.vector.tensor_scalar_mul(pkg[S], pkg[S], 0.125)
                    nc.vector.tensor_scalar_mul(m1g[S], iota_g[S], 2.0 ** -16)
                    nc.vector.tensor_tensor(pkg[S], pkg[S], m1g[S], op=mybir.AluOpType.add)
                    for jb in range(4):
                        nc.vector.transpose(pkT[ts(jb, 32), t2, :],
                                            pkg2[S, ts(jb, 32)])
                # interleave ready local-attention blocks (1-chunk lag);
                # hold back the last blocks to cover phase-B latency
                hi = min(4 * c - 2 + 1, NB - DEFER)
                for b in range(A_DONE[0], hi):
                    for h in range(NH):
                        local_block(h, b)
                A_DONE[0] = max(A_DONE[0], hi)
                if REST and c < NCHUNK - 1:
                    last = None
                    for rr in range(REST):
                        rd = dram.tile([128, 4, CW], BF16, tag="restd")
                        w = nc.gpsimd.dma_start(
                            rd[:], xt_d[c, 0:4, :, :].rearrange("ko p t -> p ko t"))
                        if last is not None:
                            add_dep_helper(_raw(w), last, reason="rest chain")
                        last = _raw(w)
                    rest_gate[0] = last

        wkv.release()

        # ---------------- phase B part 1: candidate top-72 funnel ----------------
        gp = tc.alloc_tile_pool(name="gp", bufs=1)
        gbig = tc.alloc_tile_pool(name="gbig", bufs=2)
        gw = tc.alloc_tile_pool(name="gw", bufs=2)
        # wq residual for the exact re-projection; only used in phase B so
        # loaded here (after wkv released its SBUF), overlapping the funnel
        wlo = gbig.tile([128, KO, NH * D], BF16, tag="wlo", bufs=1)
        nc.scalar.dma_start(wlo[:], wlor[:])

        # top-16 per pkT row (row = (h,j): 128 tokens {p*32+j}); top-72 of a
        # head has <=16 tokens in any such class w.h.p.
        pkT2 = pkT[:].rearrange("p t j -> p (t j)")
        m16 = gp.tile([128, 16], F32)
        nc.vector.max(out=m16[:, 0:8], in_=pkT2)
        nc.vector.match_replace(out=pkT2, in_to_replace=m16[:, 0:8],
                                in_values=pkT2, imm_value=-1e30)
        nc.vector.max(out=m16[:, 8:16], in_=pkT2)
        # regroup to one partition per head via PE transpose + DRAM bounce
        pT2 = psA2k("pT2")[:16, :128]
        nc.tensor.transpose(pT2, m16[:], ident[:])
        mTf = gp.tile([16, 128], F32)
        nc.vector.tensor_copy(mTf[:], pT2)
        mTd = dram.tile([16, 128], F32)
        w1 = nc.sync.dma_start(mTd[:], mTf[:])
        lvl3 = gp.tile([NH, 512], F32)
        r3 = nc.sync.dma_start(
            lvl3[:].rearrange("h (j r) -> h j r", j=32),
            mTd[:].rearrange("r (h j) -> h j r", h=NH))
        add_dep_helper(_raw(r3), _raw(w1), reason="lvl3 read after write")
        tops = gp.tile([NH, NCAND], F32)
        for rr in range(NCAND // 8):
            nc.vector.max(out=tops[:, ts(rr, 8)], in_=lvl3[:])
            if rr < NCAND // 8 - 1:
                nc.vector.match_replace(out=lvl3[:], in_to_replace=tops[:, ts(rr, 8)],
                                        in_values=lvl3[:], imm_value=-1e30)

        def decode_t(dst, src, n):
            t1 = gp.tile([NH, n], F32, tag="dec1")
            nc.vector.tensor_scalar_mul(t1[:], src, 8.0)
            t1i = gp.tile([NH, n], I32, tag="dec2")
            nc.vector.tensor_copy(t1i[:], t1[:])
            t1f = gp.tile([NH, n], F32, tag="dec3")
            nc.vector.tensor_copy(t1f[:], t1i[:])
            nc.vector.tensor_tensor(t1[:], t1[:], t1f[:], op=mybir.AluOpType.subtract)
            nc.vector.tensor_scalar_mul(dst, t1[:], 8192.0)

        cand_t = gp.tile([NH, NSLOT], F32)
        decode_t(cand_t[:, 0:NCAND], tops[:], NCAND)
        nc.vector.memset(cand_t[:, NCAND:NCAND + 1], 0.0)
        nc.vector.memset(cand_t[:, NCAND + 1:NSLOT], 4095.0)
        if DEBUG:
            nc.sync.dma_start(dbg["cand"], cand_t[:])

        pslt = psA2k("pslt")[:NSLOT, :NH]
        nc.tensor.transpose(pslt, cand_t[:], ident[:NH, :NH])
        ctf = gp.tile([NSLOT, NH], F32)
        nc.vector.tensor_copy(ctf[:], pslt)
        cti = gp.tile([NSLOT, NH], I32)
        nc.vector.tensor_copy(cti[:], ctf[:])

        # candidate-row gathers (hi|lo bf16 pairs) for all heads, issued
        # back-to-back so the software-DGE flights overlap; the deferred
        # local blocks keep PE busy while they land.
        xsels = []
        for h in range(NH):
            xsel = gbig.tile([NPAD, 2 * H], BF16, tag="xsel", bufs=4, name=f"xsel{h}")
            nc.gpsimd.indirect_dma_start(
                out=xsel[0:NSLOT, :], out_offset=None, in_=xhl_d,
                in_offset=bass.IndirectOffsetOnAxis(ap=cti[:, h:h + 1], axis=0))
            xsels.append(xsel)

        b0_def = A_DONE[0]
        for b in range(A_DONE[0], NB):
            if REST_B and b > b0_def and (b - b0_def) % 5 == 0:
                last = None
                for rr in range(2):
                    rdb = gw.tile([128, CW], BF16, tag="restb", bufs=2)
                    w = nc.sync.dma_start(rdb[:], xt_d[b % NCHUNK, 0, :, :])
                    if last is not None:
                        add_dep_helper(_raw(w), last, reason="rest chain B")
                    last = _raw(w)
                rest_gate[0] = last
            for h in range(NH):
                local_block(h, b)

        ne_all = gp.tile([NH, NSLOT], F32)
        qgTh = [None] * NH

        def prep_head(h):
            # exact re-projection of the candidate q rows (selection must
            # match the reference's fp32 norms bit-closely): host-split
            # bf16 hi+lo rows; q = xh@wh + xl@wh + xh@wl (xl@wl ~ 1e-6,
            # dropped). Slabs transposed by the DMA xbar (no PE cost).
            xhT = gbig.tile([128, KO, NSLOT], BF16, tag="xhT", bufs=2)
            xlT = gbig.tile([128, KO, NSLOT], BF16, tag="xlT", bufs=2)
            for half, dst in ((0, xhT), (1, xlT)):
                for kb in range(KO):
                    ptx = psum.tile([128, 1024], BF16, tag="A2k", bufs=2,
                                    name="ptx")[:, 0:NSLOT]
                    nc.tensor.transpose(
                        ptx, xsels[h][0:NSLOT, half * H + kb * 128:half * H + (kb + 1) * 128],
                        identb[:NSLOT, :NSLOT])
                    nc.vector.tensor_copy(dst[:, kb, :], ptx)
            pqc = psACC("pqc")[:, :NSLOT]
            for i, (w_, x_) in enumerate(((wqb, xhT), (wqb, xlT), (wlo, xhT))):
                for kb in range(KO):
                    nc.tensor.matmul(pqc, w_[:, kb, ts(h, D)], x_[:, kb, 0:NSLOT],
                                     start=(i == 0 and kb == 0),
                                     stop=(i == 2 and kb == KO - 1),
                                     skip_group_check=True)
            qcf = gw.tile([128, NSLOT], F32, tag="qcf")
            nc.vector.tensor_copy(qcf[:], pqc)
            qgTh[h] = gbig.tile([128, NSLOT], BF16, tag=f"qgT{h}", name=f"qgT{h}")
            nc.vector.tensor_copy(qgTh[h][:], qcf[:])
            sqc = gw.tile([128, NSLOT], F32, tag="sqc")
            nc.vector.tensor_tensor(sqc[:], qcf[:], qcf[:], op=mybir.AluOpType.mult)
            pne = psA2k("pne")[:1, :NSLOT]
            nc.tensor.matmul(pne, ones[:], sqc[:], start=True, stop=True)
            nerow = gw.tile([1, NSLOT], F32, tag="nerow")
            nc.vector.tensor_copy(nerow[:], pne)
            nc.scalar.dma_start(ne_all[h:h + 1, :], nerow[:])

        # software-pipelined: head h+1 preps while head h runs on PE
        PgTs = [None] * NH
        gcos = [None] * NH
        prep_head(0)
        PgTs[0] = global_scores(0)
        prep_head(1)
        gcos[0] = global_ctx(0, PgTs[0])
        PgTs[1] = global_scores(1)
        prep_head(2)
        gcos[1] = global_ctx(1, PgTs[1])
        PgTs[2] = global_scores(2)
        prep_head(3)
        if DEBUG:
            nc.sync.dma_start(dbg["ne"], ne_all[:])

        # threshold/selection chain (DVE; overlaps the PE work above)
        ne_work = gp.tile([NH, NSLOT], F32)
        nc.vector.tensor_copy(ne_work[:], ne_all[:])
        tops_e = gp.tile([NH, 64], F32)
        for rr in range(8):
            nc.vector.max(out=tops_e[:, ts(rr, 8)], in_=ne_work[:])
            if rr < 7:
                nc.vector.match_replace(out=ne_work[:], in_to_replace=tops_e[:, ts(rr, 8)],
                                        in_values=ne_work[:], imm_value=-1e30)
        theta = gp.tile([NH, 1], F32)
        nc.vector.tensor_copy(theta[:], tops_e[:, 61:62])

        # sel over the slots; specials (bos/eos) always selected
        sel = gp.tile([NH, NSLOT], F32)
        nc.vector.tensor_tensor(sel[:], ne_all[:], theta[:].to_broadcast([NH, NSLOT]),
                                op=mybir.AluOpType.is_ge)
        nc.vector.memset(sel[:, NCAND:NSLOT], 1.0)
        # scatter idx per slot: cand_t if selected else OOB (100000)
        sidx_f = gp.tile([NH, NSLOT], F32)
        nc.vector.tensor_scalar(sidx_f[:], sel[:], -1.0, scalar2=None,
                                op0=mybir.AluOpType.add)
        nc.vector.tensor_scalar_mul(sidx_f[:], sidx_f[:], -100000.0)
        nc.vector.tensor_tensor(sidx_f[:], sidx_f[:], cand_t[:], op=mybir.AluOpType.add)
        # fold head into the row index: row = token*NH + h (see scatter_head)
        nc.vector.tensor_scalar_mul(sidx_f[:], sidx_f[:], float(NH))
        hcol = gp.tile([NH, 1], F32)
        nc.gpsimd.iota(hcol[:], pattern=[[0, 1]], base=0, channel_multiplier=1,
                       allow_small_or_imprecise_dtypes=True)
        nc.vector.tensor_tensor(sidx_f[:], sidx_f[:],
                                hcol[:].to_broadcast([NH, NSLOT]),
                                op=mybir.AluOpType.add)
        p_ = psA2k("ptr")[:NSLOT, :NH]
        nc.tensor.transpose(p_, sidx_f[:], ident[:NH, :NH])
        sf1 = gp.tile([NSLOT, NH], F32)
        nc.vector.tensor_copy(sf1[:], p_)
        sidx_i = gp.tile([NSLOT, NH], I32)
        nc.vector.tensor_copy(sidx_i[:], sf1[:])
        if DEBUG:
            nc.sync.dma_start(dbg["sidx"], sidx_i[:])

        scatter_head(0, gcos[0])
        scatter_head(1, gcos[1])
        gcos[2] = global_ctx(2, PgTs[2])
        scatter_head(2, gcos[2])
        PgTs[3] = global_scores(3)
        gcos[3] = global_ctx(3, PgTs[3])
        scatter_head(3, gcos[3])

        gw.release()
        gbig.release()
        gp.release()
        wkv2.release()
        ab.release()
        psum.release()
        dram.release()
        res.release()
        const.release()

    nc.finalize()
    return nc


_NC_CACHE = None


def make_in_maps(inputs):
    import ml_dtypes
    BF = ml_dtypes.bfloat16
    hs = np.ascontiguousarray(np.asarray(inputs["hidden_states"], dtype=np.float32))
    Wq = np.ascontiguousarray(np.asarray(inputs["Wq"], dtype=np.float32))
    Wk = np.ascontiguousarray(np.asarray(inputs["Wk"], dtype=np.float32))
    Wv = np.ascontiguousarray(np.asarray(inputs["Wv"], dtype=np.float32))
    ident = np.eye(128, dtype=np.float32)
    # chunk-major [c, ko, p, t] layout (contiguous per-chunk slabs)
    xts_host = [
        np.ascontiguousarray(
            hs[n].T.astype(BF).reshape(16, 128, 8, 512).transpose(2, 0, 1, 3))
        for n in range(2)
    ]
    # hi|lo bf16 row pairs for the exact-norm gathers; the DMA-xbar
    # transpose lands hidden row r at (partition r%128, slab r//128),
    # matching the standard weight-slab convention.
    xhl_host = []
    for n in range(2):
        xh = hs[n].astype(BF)
        xl = (hs[n] - xh.astype(np.float32)).astype(BF)
        xhl_host.append(np.ascontiguousarray(
            np.concatenate([xh, xl], axis=1)))  # [T, 2H]
    Wq_bf = Wq.astype(BF)
    Wq_lo = (Wq - Wq_bf.astype(np.float32)).astype(BF)
    in_maps = []
    for c in range(8):
        n = c // 4
        h0 = (c % 4) * NH
        cols = slice(h0 * D, (h0 + NH) * D)
        in_maps.append({
            "xt": xts_host[n],
            "xhl": xhl_host[n],
            "wq": np.ascontiguousarray(Wq_bf[:, cols]),
            "wk": np.ascontiguousarray(Wk[:, cols].astype(BF)),
            "wv": np.ascontiguousarray(Wv[:, cols].astype(BF)),
            "wlo": np.ascontiguousarray(Wq_lo[:, cols]),
            "ident": ident,
            "identb": ident.astype(BF),
        })
    return in_maps


def kernel(**inputs):
    global _NC_CACHE
    if _NC_CACHE is None:
        _NC_CACHE = build_program()
    nc = _NC_CACHE
    in_maps = make_in_maps(inputs)
    res = run_bass_kernel_spmd(nc, in_maps, core_ids=list(range(8)))
    out = np.zeros((2, T, H), np.float32)
    for c in range(8):
        n = c // 4
        h0 = (c % 4) * NH
        out[n, :, h0 * D:(h0 + NH) * D] = res.results[c]["out"]
    return out


# revision 20
# speedup vs baseline: 1.0495x; 1.0023x over previous
"""Block-global self-attention Trainium2 kernel (SPMD over 8 NeuronCores).

Sharding: core c -> batch n = c//4, heads h0 = (c%4)*4 .. h0+3.
Each core receives xt = hidden[n].T (bf16, chunk-major) and wq/wk/wv =
W[:, cols] [2048,512] bf16, returns out [4096,512] (its head-column
stripe of batch n).

Per-core pipeline:
  P: bf16 projections (direct DMA of host-side bf16 xt/weights) -> qT/kT
     [d,t] + V2 (t-major, 64-row-shifted so local windows are two aligned
     full-K tiles). Approx q-norms land in a [128,NH,32] grid per chunk
     (DRAM bounce per chunk); the packed value (quantized norm + token id
     in low mantissa) transform + a DVE 32x32 stream-transpose into pkT
     [(h,j), pair, p] run incrementally under the chunk loop. Local
     blocks interleave with a 1-chunk lag; the last DEFER blocks are
     deferred to cover phase-B latency.
  A: local block attention; softmax without max-subtraction (|score|<8);
     probs kept unnormalized bf16, 1/denom fused into the final ACT copy.
  B: top-16 per pkT row (128-token classes) -> PE transpose + one DRAM
     bounce -> per-head pools [4,512] -> 9 max8/match_replace rounds ->
     top-72 candidates + bos/eos. Four indirect row gathers (hi|lo bf16
     pairs from host-packed xhl) issue back-to-back and overlap the
     deferred local blocks. Exact fp32 candidate q (hi/lo trick, wlo
     host-precomputed) via DMA-transposed slabs (no PE transposes);
     global attention transposed over all 74 slots; exact top-62
     threshold picks the final set; per-head scatters roll out as soon
     as selection + that head's ctx are ready.
"""
import os
import numpy as np

import concourse.bass as bass
import concourse.bacc as bacc
import concourse.mybir as mybir
from concourse.tile import TileContext, add_dep_helper
from concourse.bass_utils import run_bass_kernel_spmd

F32 = mybir.dt.float32
BF16 = mybir.dt.bfloat16
I32 = mybir.dt.int32

T = 4096
H = 2048
D = 128
NH = 4
KO = H // 128
NB = T // 128
CW = 512
NCHUNK = T // CW
NEG = -30.0
SCALE = float(1.0 / np.sqrt(128.0))
NCAND = 72
NSLOT = NCAND + 2
NPAD = 80          # gather/transpose row padding (xbar needs %16)
GEXP = 512 // NSLOT  # global score blocks per psum bank / exp call
DEBUG = bool(int(os.environ.get("KERNEL_DEBUG", "0")))
# PE rest: chained delay-DMAs per chunk boundary; breaks the sustained
# PE-activity streak that trips the P0 power-state downclock
REST = int(os.environ.get("KERNEL_REST", "1"))
DEFER = int(os.environ.get("KERNEL_DEFER", "24"))
REST_B = int(os.environ.get("KERNEL_REST_B", "0"))


def ts(i, sz):
    return slice(i * sz, (i + 1) * sz)


def _raw(inst):
    return inst.ins if hasattr(inst, "ins") else inst


def build_program():
    nc = bacc.Bacc("TRN2", target_bir_lowering=False, debug=False,
                   enable_asserts=True)
    # chunk-major xt layout: [c, ko, p, t] so each chunk DMA reads a
    # contiguous 512KB slab (sequential DRAM >> strided)
    xt_d = nc.dram_tensor("xt", (NCHUNK, KO, 128, CW), BF16,
                          kind="ExternalInput").ap()
    xhl_d = nc.dram_tensor("xhl", (T, 2 * H), BF16, kind="ExternalInput").ap()
    wq_d = nc.dram_tensor("wq", (H, NH * D), BF16, kind="ExternalInput").ap()
    wk_d = nc.dram_tensor("wk", (H, NH * D), BF16, kind="ExternalInput").ap()
    wv_d = nc.dram_tensor("wv", (H, NH * D), BF16, kind="ExternalInput").ap()
    wlo_d = nc.dram_tensor("wlo", (H, NH * D), BF16, kind="ExternalInput").ap()
    id_d = nc.dram_tensor("ident", (128, 128), F32, kind="ExternalInput").ap()
    idb_d = nc.dram_tensor("identb", (128, 128), BF16, kind="ExternalInput").ap()
    out_d = nc.dram_tensor("out", (T, NH * D), F32, kind="ExternalOutput").ap()
    dbg = {}
    if DEBUG:
        dbg["na"] = nc.dram_tensor("dbg_na", (128, NH, 32), F32, kind="ExternalOutput").ap()
        dbg["cand"] = nc.dram_tensor("dbg_cand", (NH, NSLOT), F32, kind="ExternalOutput").ap()
        dbg["ne"] = nc.dram_tensor("dbg_ne", (NH, NSLOT), F32, kind="ExternalOutput").ap()
        dbg["sidx"] = nc.dram_tensor("dbg_sidx", (NSLOT, NH), I32, kind="ExternalOutput").ap()

    with TileContext(nc) as tc:
        const = tc.alloc_tile_pool(name="const", bufs=1)
        res = tc.alloc_tile_pool(name="res", bufs=1)
        dram = tc.alloc_tile_pool(name="dram", bufs=1, space="DRAM")

        ident = const.tile([128, 128], F32)
        nc.sync.dma_start(ident[:], id_d)
        identb = const.tile([128, 128], BF16)
        nc.sync.dma_start(identb[:], idb_d)
        ones_b = const.tile([128, 1], BF16)
        nc.vector.memset(ones_b[:], 1.0)
        ones = const.tile([128, 1], F32)
        nc.vector.memset(ones[:], 1.0)
        iota_g = const.tile([128, NH, 32], F32)
        nc.gpsimd.iota(iota_g[:], pattern=[[0, NH], [1, 32]], base=0,
                       channel_multiplier=32, allow_small_or_imprecise_dtypes=True)
        kT = [res.tile([128, 64 + T + 64], BF16, tag=f"kT{h}", name=f"kT{h}") for h in range(NH)]
        V2 = res.tile([128, NB + 1, NH, D + 1], BF16, tag="V2")
        pkT = res.tile([128, NCHUNK // 2, 32], F32, tag="pkT")
        wqb = res.tile([128, KO, NH * D], BF16, tag="wqb")
        na_dram = dram.tile([NH, T], F32)

        # ---------------- pools ----------------
        psum = tc.alloc_tile_pool(name="psum", bufs=1, space="PSUM")
        ab = tc.alloc_tile_pool(name="ab", bufs=4)

        def psA2k(nm):   # 2KB f32 one-shot psums
            t = psum.tile([128, 512], F32, tag="A2k", bufs=2, name=nm)
            return t
        def psBLK(nm):   # per-block S + ctx combined
            t = psum.tile([128, 512], F32, tag="blk", bufs=2, name=nm)
            return t
        def psSG(nm):    # global score groups
            t = psum.tile([128, 512], F32, tag="psg", bufs=2, name=nm)
            return t
        def psACC(nm):   # held accumulators
            t = psum.tile([128, 512], F32, tag="ACC", bufs=2, name=nm)
            return t

        # ---------------- interleaved: local attention + global per head ----------------
        out_write_insts = []
        cur_co = [None]
        rest_gate = [None]

        def local_block(h, b):
            blk = psBLK("blk")
            # S^T halves: [tk(128), tq(128)]; half g covers window pos g*128..,
            # i.e. k tokens [b*128 - 64 + g*128, ...). kT is 64-padded.
            for g in range(2):
                seg = b + g
                mi = nc.tensor.matmul(blk[:, g * 128:(g + 1) * 128],
                                 kT[h][:, seg * 128:seg * 128 + 128],
                                 qT[h][:, ts(b, 128)], start=True, stop=True)
                if rest_gate[0] is not None:
                    add_dep_helper(_raw(mi), rest_gate[0], reason="rest gate")
                    rest_gate[0] = None
            PT = ab.tile([128, 256], BF16, tag="PT", name="PT", bufs=2)
            nc.scalar.activation(PT[:], blk[:, 0:256], mybir.ActivationFunctionType.Exp,
                                 scale=SCALE)
            pC = blk[:, 256:385]
            nc.tensor.matmul(pC, PT[:, 0:128], V2[:, b, h, :],
                             start=True, stop=False)
            nc.tensor.matmul(pC, PT[:, 128:256], V2[:, b + 1, h, :],
                             start=False, stop=True)
            rc = ab.tile([128, 1], F32, tag="rc", name="rc", bufs=8)
            nc.vector.reciprocal(rc[:], pC[:, 128:129])
            # all 4 heads of a block share one staging tile -> one 256KB
            # out write with 2KB rows (descriptor-rate-bound: 4x fewer DMAs)
            if h == 0:
                cur_co[0] = ab.tile([128, NH, D], F32, tag="co4", name="co4",
                                    bufs=2)
            nc.scalar.activation(cur_co[0][:, h, :], pC[:, 0:D],
                                 mybir.ActivationFunctionType.Copy, scale=rc[:])
            if h == NH - 1:
                w = nc.sync.dma_start(
                    out_d[ts(b, 128), :],
                    cur_co[0][:].rearrange("p h d -> p (h d)"))
                out_write_insts.append(_raw(w))

        def global_scores(h):
            # SgT blocks: psum [t(128), slot]; block jj covers tokens
            # jj*128-64 .. jj*128+63 (kT cols jj*128..+128, V2 block jj).
            # Pad tokens give exp(0)=1 but V2 values AND ones-col are 0
            # there, so they contribute nothing.
            PgT = gbig.tile([128, NB + 1, NSLOT], BF16, tag="PgT",
                            name=f"PgT{h}", bufs=2)
            jj = 0
            while jj <= NB:
                nb = min(GEXP, NB + 1 - jj)
                psg = psSG("psg")
                for gi in range(nb):
                    nc.tensor.matmul(psg[:, gi * NSLOT:(gi + 1) * NSLOT],
                                     kT[h][:, (jj + gi) * 128:(jj + gi + 1) * 128],
                                     qgTh[h][:], start=True, stop=True)
                nc.scalar.activation(
                    PgT[:, jj:jj + nb, :],
                    psg[:, 0:nb * NSLOT].rearrange("p (b s) -> p b s", b=nb),
                    mybir.ActivationFunctionType.Exp, scale=SCALE)
                jj += nb
            return PgT

        def global_ctx(h, PgT):
            pgc = psACC("pgc")[:NSLOT, :D + 1]
            for jj in range(NB + 1):
                nc.tensor.matmul(pgc, PgT[:, jj, :], V2[:, jj, h, :],
                                 start=(jj == 0), stop=(jj == NB),
                                 skip_group_check=True)
            rcg = gw.tile([NSLOT, 1], F32, tag="rcg", bufs=4)
            nc.vector.reciprocal(rcg[:], pgc[:, D:D + 1])
            gco = gw.tile([NSLOT, 128], F32, tag="gco", bufs=4)
            nc.scalar.activation(gco[:], pgc[:, 0:D],
                                 mybir.ActivationFunctionType.Copy, scale=rcg[:])
            return gco

        def scatter_head(h, gco):
            # out viewed as [T*NH, D] rows; sidx encodes token*NH + h so the
            # out AP keeps offset 0 (DynamicAP requirement)
            scat = nc.gpsimd.indirect_dma_start(
                out=out_d.rearrange("t (h d) -> (t h) d", h=NH),
                out_offset=bass.IndirectOffsetOnAxis(ap=sidx_i[:, h:h + 1], axis=0),
                in_=gco[:], in_offset=None,
                bounds_check=T * NH - 1, oob_is_err=False)
            for w in out_write_insts:
                add_dep_helper(_raw(scat), w, reason="scatter after local writes")


        A_DONE = [0]
        # ---------------- phase P ----------------
        wkv2 = tc.alloc_tile_pool(name="wkv2", bufs=1)
        wkv = tc.alloc_tile_pool(name="wkv", bufs=1)
        qT = [wkv2.tile([128, T], BF16, tag=f"qT{h}", name=f"qT{h}") for h in range(NH)]
        wkb = wkv.tile([128, KO, NH * D], BF16, tag="wkb")
        wvb = wkv.tile([128, KO, NH * D], BF16, tag="wvb")
        wb = {"q": wqb, "k": wkb, "v": wvb}

        with tc.tile_pool(name="pp", bufs=2) as pp, \
             tc.tile_pool(name="pp1", bufs=1) as pp1:

            xtb_tiles = {}

            def load_xtb(c):
                t = pp1.tile([128, KO, CW], BF16, tag="xtb", bufs=2)
                for kg in range(4):
                    nc.gpsimd.dma_start(
                        t[:, kg * 4:(kg + 1) * 4, :],
                        xt_d[c, kg * 4:(kg + 1) * 4, :, :].rearrange("ko p t -> p ko t"))
                return t

            # ramp order on the Pool queue: wq -> x chunk 0 -> wk -> wv
            # (queues serialize at the DMA arbiter, so issue in need-order)
            wrs = {nm: wd.rearrange("(ko p) m -> p ko m", p=128)
                   for nm, wd in (("q", wq_d), ("k", wk_d), ("v", wv_d))}
            wlor = wlo_d.rearrange("(ko p) m -> p ko m", p=128)
            nc.gpsimd.dma_start(wb["q"][:], wrs["q"][:])
            xtb_tiles[0] = load_xtb(0)
            for nm in ("k", "v"):
                nc.gpsimd.dma_start(wb[nm][:], wrs[nm][:])

            for h in range(NH):
                nc.vector.memset(kT[h][:, 0:64], 0.0)
                nc.vector.memset(kT[h][:, 64 + T:], 0.0)
            nc.vector.memset(V2[0:64, 0, :, :], 0.0)
            nc.vector.memset(V2[64:128, NB, :, :], 0.0)
            nc.vector.memset(V2[:, :, :, D:D + 1], 1.0)
            # pad rows contribute neither value nor denominator mass
            nc.vector.memset(V2[0:64, 0, :, D:D + 1], 0.0)
            nc.vector.memset(V2[64:128, NB, :, D:D + 1], 0.0)

            # packed-value transform scratch (slab-sliced per chunk)
            m0g = pp.tile([128, NH, 32], F32, tag="m0g", bufs=1)
            m1g = pp.tile([128, NH, 32], F32, tag="m1g", bufs=1)
            nagpg = pp.tile([128, NH, 32], F32, tag="nagpg", bufs=1)
            pkg = pp.tile([128, NH, 32], F32, tag="pkg", bufs=1)
            pkig = pp.tile([128, NH, 32], I32, tag="pkig", bufs=1)
            pkg2 = pkg[:].rearrange("p h j -> p (h j)")

            for c in range(NCHUNK):
                xtb = xtb_tiles.pop(c) if c in xtb_tiles else load_xtb(c)
                na_chunk_writes = []
                for h in range(NH):
                    for nm, dstT in (("q", qT[h]), ("k", kT[h])):
                        ps = psA2k("psqk")
                        for kb in range(KO):
                            mi = nc.tensor.matmul(ps[:], wb[nm][:, kb, ts(h, D)],
                                                  xtb[:, kb, :], start=(kb == 0),
                                                  stop=(kb == KO - 1))
                            if rest_gate[0] is not None:
                                add_dep_helper(_raw(mi), rest_gate[0],
                                               reason="PE rest gate")
                                rest_gate[0] = None
                        off = 64 if nm == "k" else 0
                        nc.vector.tensor_copy(dstT[:, off + c * CW:off + (c + 1) * CW], ps[:])
                        if nm == "q":
                            sq = pp.tile([128, CW], BF16, tag="sq", bufs=1)
                            nc.vector.tensor_tensor(sq[:], dstT[:, ts(c, CW)],
                                                    dstT[:, ts(c, CW)],
                                                    op=mybir.AluOpType.mult)
                            pn = psA2k("pn")[:1, :]
                            nc.tensor.matmul(pn, ones_b[:], sq[:],
                                             start=True, stop=True)
                            narow = pp.tile([1, CW], F32, tag="narow", bufs=1)
                            nc.vector.tensor_copy(narow[:], pn)
                            w = nc.sync.dma_start(na_dram[h:h + 1, ts(c, CW)], narow[:])
                            na_chunk_writes.append(_raw(w))
                for s in range(CW // 128):
                    sg = c * (CW // 128) + s
                    pv = psA2k("psv")
                    for kb in range(KO):
                        nc.tensor.matmul(pv[:], xtb[:, kb, ts(s, 128)],
                                         wb["v"][:, kb, :], start=(kb == 0),
                                         stop=(kb == KO - 1))
                    vt = pp.tile([128, NH * D], BF16, tag="vtmp", bufs=1)
                    nc.vector.tensor_copy(vt[:], pv[:])
                    nc.sync.dma_start(V2[64:128, sg, :, 0:D],
                                      vt[0:64, :].rearrange("p (h d) -> p h d", h=NH))
                    nc.sync.dma_start(V2[0:64, sg + 1, :, 0:D],
                                      vt[64:128, :].rearrange("p (h d) -> p h d", h=NH))
                # incremental norm grid + packed transform for this chunk
                # (tokens c*512.. live on grid partitions c*16..c*16+16)
                r = nc.sync.dma_start(
                    nagpg[ts(c, 16), :, :],
                    na_dram[:, ts(c, CW)].rearrange("h (p j) -> p h j", p=16))
                for w in na_chunk_writes:
                    add_dep_helper(_raw(r), w, reason="na slab read after writes")
                if c % 2 == 1:
                    # DVE partition offsets are quadrant-granular: transform
                    # the finished 32-partition chunk pair, then
                    # stream-transpose it into pkT rows (h*32+j)
                    t2 = c // 2
                    S = slice(t2 * 32, (t2 + 1) * 32)
                    nc.vector.tensor_scalar(m0g[S], iota_g[S], 0.0, scalar2=None,
                                            op0=mybir.AluOpType.is_equal)
                    nc.vector.tensor_scalar(m1g[S], iota_g[S], 4095.0, scalar2=None,
                                            op0=mybir.AluOpType.is_equal)
                    nc.vector.tensor_tensor(m0g[S], m0g[S], m1g[S], op=mybir.AluOpType.add)
                    nc.vector.tensor_tensor(m1g[S], nagpg[S], m0g[S], op=mybir.AluOpType.mult)
                    nc.vector.tensor_tensor(nagpg[S], nagpg[S], m1g[S], op=mybir.AluOpType.subtract)
                    nc.vector.tensor_scalar_mul(m0g[S], m0g[S], 1.0e6)
                    nc.vector.tensor_tensor(nagpg[S], nagpg[S], m0g[S], op=mybir.AluOpType.subtract)
                    nc.vector.tensor_scalar_mul(pkg[S], nagpg[S], 4.0)
                    nc.vector.tensor_copy(pkig[S], pkg[S])
                    nc.vector.tensor_copy(pkg[S], pkig[S])
                    nc.vector.tensor_scalar_mul(pkg[S], pkg[S], 0.125)
                    nc.vector.tensor_scalar_mul(m1g[S], iota_g[S], 2.0 ** -16)
                    nc.vector.tensor_tensor(pkg[S], pkg[S], m1g[S], op=mybir.AluOpType.add)
                    for jb in range(4):
                        nc.vector.transpose(pkT[ts(jb, 32), t2, :],
                                            pkg2[S, ts(jb, 32)])
                # interleave ready local-attention blocks (1-chunk lag);
                # hold back the last blocks to cover phase-B latency
                hi = min(4 * c - 2 + 1, NB - DEFER)
                for b in range(A_DONE[0], hi):
                    for h in range(NH):
                        local_block(h, b)
                A_DONE[0] = max(A_DONE[0], hi)
                if REST and c < NCHUNK - 1:
                    last = None
                    for rr in range(REST):
                        rd = dram.tile([128, 4, CW], BF16, tag="restd")
                        w = nc.gpsimd.dma_start(
                            rd[:], xt_d[c, 0:4, :, :].rearrange("ko p t -> p ko t"))
                        if last is not None:
                            add_dep_helper(_raw(w), last, reason="rest chain")
                        last = _raw(w)
                    rest_gate[0] = last

        wkv.release()

        # ---------------- phase B part 1: candidate top-72 funnel ----------------
        gp = tc.alloc_tile_pool(name="gp", bufs=1)
        gbig = tc.alloc_tile_pool(name="gbig", bufs=2)
        gw = tc.alloc_tile_pool(name="gw", bufs=2)
        # wq residual for the exact re-projection; only used in phase B so
        # loaded here (after wkv released its SBUF), overlapping the funnel
        wlo = gbig.tile([128, KO, NH * D], BF16, tag="wlo", bufs=1)
        nc.scalar.dma_start(wlo[:], wlor[:])

        # top-16 per pkT row (row = (h,j): 128 tokens {p*32+j}); top-72 of a
        # head has <=16 tokens in any such class w.h.p.
        pkT2 = pkT[:].rearrange("p t j -> p (t j)")
        m16 = gp.tile([128, 16], F32)
        nc.vector.max(out=m16[:, 0:8], in_=pkT2)
        nc.vector.match_replace(out=pkT2, in_to_replace=m16[:, 0:8],
                                in_values=pkT2, imm_value=-1e30)
        nc.vector.max(out=m16[:, 8:16], in_=pkT2)
        # regroup to one partition per head via PE transpose + DRAM bounce
        pT2 = psA2k("pT2")[:16, :128]
        nc.tensor.transpose(pT2, m16[:], ident[:])
        mTf = gp.tile([16, 128], F32)
        nc.vector.tensor_copy(mTf[:], pT2)
        mTd = dram.tile([16, 128], F32)
        w1 = nc.sync.dma_start(mTd[:], mTf[:])
        lvl3 = gp.tile([NH, 512], F32)
        r3 = nc.sync.dma_start(
            lvl3[:].rearrange("h (j r) -> h j r", j=32),
            mTd[:].rearrange("r (h j) -> h j r", h=NH))
        add_dep_helper(_raw(r3), _raw(w1), reason="lvl3 read after write")
        tops = gp.tile([NH, NCAND], F32)
        for rr in range(NCAND // 8):
            nc.vector.max(out=tops[:, ts(rr, 8)], in_=lvl3[:])
            if rr < NCAND // 8 - 1:
                nc.vector.match_replace(out=lvl3[:], in_to_replace=tops[:, ts(rr, 8)],
                                        in_values=lvl3[:], imm_value=-1e30)

        def decode_t(dst, src, n):
            t1 = gp.tile([NH, n], F32, tag="dec1")
            nc.vector.tensor_scalar_mul(t1[:], src, 8.0)
            t1i = gp.tile([NH, n], I32, tag="dec2")
            nc.vector.tensor_copy(t1i[:], t1[:])
            t1f = gp.tile([NH, n], F32, tag="dec3")
            nc.vector.tensor_copy(t1f[:], t1i[:])
            nc.vector.tensor_tensor(t1[:], t1[:], t1f[:], op=mybir.AluOpType.subtract)
            nc.vector.tensor_scalar_mul(dst, t1[:], 8192.0)

        cand_t = gp.tile([NH, NSLOT], F32)
        decode_t(cand_t[:, 0:NCAND], tops[:], NCAND)
        nc.vector.memset(cand_t[:, NCAND:NCAND + 1], 0.0)
        nc.vector.memset(cand_t[:, NCAND + 1:NSLOT], 4095.0)
        if DEBUG:
            nc.sync.dma_start(dbg["cand"], cand_t[:])

        pslt = psA2k("pslt")[:NSLOT, :NH]
        nc.tensor.transpose(pslt, cand_t[:], ident[:NH, :NH])
        ctf = gp.tile([NSLOT, NH], F32)
        nc.vector.tensor_copy(ctf[:], pslt)
        cti = gp.tile([NSLOT, NH], I32)
        nc.vector.tensor_copy(cti[:], ctf[:])

        # candidate-row gathers (hi|lo bf16 pairs) for all heads, issued
        # back-to-back so the software-DGE flights overlap; the deferred
        # local blocks keep PE busy while they land.
        xsels = []
        for h in range(NH):
            xsel = gbig.tile([NPAD, 2 * H], BF16, tag="xsel", bufs=4, name=f"xsel{h}")
            nc.gpsimd.indirect_dma_start(
                out=xsel[0:NSLOT, :], out_offset=None, in_=xhl_d,
                in_offset=bass.IndirectOffsetOnAxis(ap=cti[:, h:h + 1], axis=0))
            xsels.append(xsel)

        b0_def = A_DONE[0]
        for b in range(A_DONE[0], NB):
            if REST_B and b > b0_def and (b - b0_def) % 5 == 0:
                last = None
                for rr in range(2):
                    rdb = gw.tile([128, CW], BF16, tag="restb", bufs=2)
                    w = nc.sync.dma_start(rdb[:], xt_d[b % NCHUNK, 0, :, :])
                    if last is not None:
                        add_dep_helper(_raw(w), last, reason="rest chain B")
                    last = _raw(w)
                rest_gate[0] = last
            for h in range(NH):
                local_block(h, b)

        ne_all = gp.tile([NH, NSLOT], F32)
        qgTh = [None] * NH

        def prep_head(h):
            # exact re-projection of the candidate q rows (selection must
            # match the reference's fp32 norms bit-closely): host-split
            # bf16 hi+lo rows; q = xh@wh + xl@wh + xh@wl (xl@wl ~ 1e-6,
            # dropped). Slabs transposed by the DMA xbar (no PE cost).
            xhT = gbig.tile([128, KO, NSLOT], BF16, tag="xhT", bufs=2)
            xlT = gbig.tile([128, KO, NSLOT], BF16, tag="xlT", bufs=2)
            for half, dst in ((0, xhT), (1, xlT)):
                for kb in range(KO):
                    ptx = psum.tile([128, 1024], BF16, tag="A2k", bufs=2,
                                    name="ptx")[:, 0:NSLOT]
                    nc.tensor.transpose(
                        ptx, xsels[h][0:NSLOT, half * H + kb * 128:half * H + (kb + 1) * 128],
                        identb[:NSLOT, :NSLOT])
                    nc.vector.tensor_copy(dst[:, kb, :], ptx)
            pqc = psACC("pqc")[:, :NSLOT]
            for i, (w_, x_) in enumerate(((wqb, xhT), (wqb, xlT), (wlo, xhT))):
                for kb in range(KO):
                    nc.tensor.matmul(pqc, w_[:, kb, ts(h, D)], x_[:, kb, 0:NSLOT],
                                     start=(i == 0 and kb == 0),
                                     stop=(i == 2 and kb == KO - 1),
                                     skip_group_check=True)
            qcf = gw.tile([128, NSLOT], F32, tag="qcf")
            nc.vector.tensor_copy(qcf[:], pqc)
            qgTh[h] = gbig.tile([128, NSLOT], BF16, tag=f"qgT{h}", name=f"qgT{h}")
            nc.vector.tensor_copy(qgTh[h][:], qcf[:])
            sqc = gw.tile([128, NSLOT], F32, tag="sqc")
            nc.vector.tensor_tensor(sqc[:], qcf[:], qcf[:], op=mybir.AluOpType.mult)
            pne = psA2k("pne")[:1, :NSLOT]
            nc.tensor.matmul(pne, ones[:], sqc[:], start=True, stop=True)
            nerow = gw.tile([1, NSLOT], F32, tag="nerow")
            nc.vector.tensor_copy(nerow[:], pne)
            nc.scalar.dma_start(ne_all[h:h + 1, :], nerow[:])

        # software-pipelined: head h+1 preps while head h runs on PE
        PgTs = [None] * NH
        gcos = [None] * NH
        prep_head(0)
        PgTs[0] = global_scores(0)
        prep_head(1)
        gcos[0] = global_ctx(0, PgTs[0])
        PgTs[1] = global_scores(1)
        prep_head(2)
        gcos[1] = global_ctx(1, PgTs[1])
        PgTs[2] = global_scores(2)
        prep_head(3)
        if DEBUG:
            nc.sync.dma_start(dbg["ne"], ne_all[:])

        # threshold/selection chain (DVE; overlaps the PE work above)
        ne_work = gp.tile([NH, NSLOT], F32)
        nc.vector.tensor_copy(ne_work[:], ne_all[:])
        tops_e = gp.tile([NH, 64], F32)
        for rr in range(8):
            nc.vector.max(out=tops_e[:, ts(rr, 8)], in_=ne_work[:])
            if rr < 7:
                nc.vector.match_replace(out=ne_work[:], in_to_replace=tops_e[:, ts(rr, 8)],
                                        in_values=ne_work[:], imm_value=-1e30)
        theta = gp.tile([NH, 1], F32)
        nc.vector.tensor_copy(theta[:], tops_e[:, 61:62])

        # sel over the slots; specials (bos/eos) always selected
        sel = gp.tile([NH, NSLOT], F32)
        nc.vector.tensor_tensor(sel[:], ne_all[:], theta[:].to_broadcast([NH, NSLOT]),
                                op=mybir.AluOpType.is_ge)
        nc.vector.memset(sel[:, NCAND:NSLOT], 1.0)
        # scatter idx per slot: cand_t if selected else OOB (100000)
        sidx_f = gp.tile([NH, NSLOT], F32)
        nc.vector.tensor_scalar(sidx_f[:], sel[:], -1.0, scalar2=None,
                                op0=mybir.AluOpType.add)
        nc.vector.tensor_scalar_mul(sidx_f[:], sidx_f[:], -100000.0)
        nc.vector.tensor_tensor(sidx_f[:], sidx_f[:], cand_t[:], op=mybir.AluOpType.add)
        # fold head into the row index: row = token*NH + h (see scatter_head)
        nc.vector.tensor_scalar_mul(sidx_f[:], sidx_f[:], float(NH))
        hcol = gp.tile([NH, 1], F32)
        nc.gpsimd.iota(hcol[:], pattern=[[0, 1]], base=0, channel_multiplier=1,
                       allow_small_or_imprecise_dtypes=True)
        nc.vector.tensor_tensor(sidx_f[:], sidx_f[:],
                                hcol[:].to_broadcast([NH, NSLOT]),
                                op=mybir.AluOpType.add)
        p_ = psA2k("ptr")[:NSLOT, :NH]
        nc.tensor.transpose(p_, sidx_f[:], ident[:NH, :NH])
        sf1 = gp.tile([NSLOT, NH], F32)
        nc.vector.tensor_copy(sf1[:], p_)
        sidx_i = gp.tile([NSLOT, NH], I32)
        nc.vector.tensor_copy(sidx_i[:], sf1[:])
        if DEBUG:
            nc.sync.dma_start(dbg["sidx"], sidx_i[:])

        scatter_head(0, gcos[0])
        scatter_head(1, gcos[1])
        gcos[2] = global_ctx(2, PgTs[2])
        scatter_head(2, gcos[2])
        PgTs[3] = global_scores(3)
        gcos[3] = global_ctx(3, PgTs[3])
        scatter_head(3, gcos[3])

        gw.release()
        gbig.release()
        gp.release()
        wkv2.release()
        ab.release()
        psum.release()
        dram.release()
        res.release()
        const.release()

    nc.finalize()
    return nc


_NC_CACHE = None


def make_in_maps(inputs):
    import ml_dtypes
    BF = ml_dtypes.bfloat16
    hs = np.ascontiguousarray(np.asarray(inputs["hidden_states"], dtype=np.float32))
    Wq = np.ascontiguousarray(np.asarray(inputs["Wq"], dtype=np.float32))
    Wk = np.ascontiguousarray(np.asarray(inputs["Wk"], dtype=np.float32))
    Wv = np.ascontiguousarray(np.asarray(inputs["Wv"], dtype=np.float32))
    ident = np.eye(128, dtype=np.float32)
    # chunk-major [c, ko, p, t] layout (contiguous per-chunk slabs)
    xts_host = [
        np.ascontiguousarray(
            hs[n].T.astype(BF).reshape(16, 128, 8, 512).transpose(2, 0, 1, 3))
        for n in range(2)
    ]
    # hi|lo bf16 row pairs for the exact-norm gathers; the DMA-xbar
    # transpose lands hidden row r at (partition r%128, slab r//128),
    # matching the standard weight-slab convention.
    xhl_host = []
    for n in range(2):
        xh = hs[n].astype(BF)
        xl = (hs[n] - xh.astype(np.float32)).astype(BF)
        xhl_host.append(np.ascontiguousarray(
            np.concatenate([xh, xl], axis=1)))  # [T, 2H]
    Wq_bf = Wq.astype(BF)
    Wq_lo = (Wq - Wq_bf.astype(np.float32)).astype(BF)
    in_maps = []
    for c in range(8):
        n = c // 4
        h0 = (c % 4) * NH
        cols = slice(h0 * D, (h0 + NH) * D)
        in_maps.append({
            "xt": xts_host[n],
            "xhl": xhl_host[n],
            "wq": np.ascontiguousarray(Wq_bf[:, cols]),
            "wk": np.ascontiguousarray(Wk[:, cols].astype(BF)),
            "wv": np.ascontiguousarray(Wv[:, cols].astype(BF)),
            "wlo": np.ascontiguousarray(Wq_lo[:, cols]),
            "ident": ident,
            "identb": ident.astype(BF),
        })
    return in_maps


def kernel(**inputs):
    global _NC_CACHE
    if _NC_CACHE is None:
        _NC_CACHE = build_program()
    nc = _NC_CACHE
    in_maps = make_in_maps(inputs)
    res = run_bass_kernel_spmd(nc, in_maps, core_ids=list(range(8)))
    out = np.zeros((2, T, H), np.float32)
    for c in range(8):
        n = c // 4
        h0 = (c % 4) * NH
        out[n, :, h0 * D:(h0 + NH) * D] = res.results[c]["out"]
    return out
